# revision 18
# baseline (speedup 1.0000x reference)
"""Trainium2 Bass kernel for AtomWise GNN message passing.

reference:
    rbf_filter = rbf @ w_rbf.T + b_rbf        # [E, C]
    msg = rbf_filter * x                      # [E, C]
    out = segment_sum(msg, edge_index_0, N)   # [N, C]
    out = silu(out @ w1.T + b1); out = silu(out @ w2.T + b2); out = out @ w3.T + b3

Strategy (8 NeuronCores, no collectives):
  - Host: stable-sort edges by destination atom; shard ATOMS (N/8 per core) so
    each core owns all edges of its atom range.  Within a core, atoms are
    processed in 128-atom tiles; each tile's edge list is padded to a global
    E_TILE so every core runs the identical SPMD program.
  - Device (per core, per 512-edge group):
      PE:  filter = rbf_chunk(K=17, bias row folded) @ w_rbfT -> PSUM
           (4 chunks packed into row-groups via tile_position, running
            concurrently on 32-row strips of the PE array)
      ACT: evacuate filter PSUM -> SBUF bf16
      DVE: msg = filter * x  (bf16 2x mode)
      DVE: one-hot[e, a] = (iota_row == li[e])  (tensor_scalar is_equal, 4x)
      PE:  atom_psum[a, c] += one-hot.T @ msg   (scatter-add as matmul)
    Then per-atom-tile PSUM -> SBUF, PE transposes to [C, atoms] layout and a
    3-layer MLP (bf16 matmuls, f32 accumulate) runs on-chip; output [1, atoms].
"""

import numpy as np

import concourse.bacc as bacc
import concourse.mybir as mybir
import concourse.tile as tile
from concourse.bass_utils import run_bass_kernel_spmd
from concourse.masks import make_identity

N_CORES = 8
P = 128
C = 256
RBF = 16
KF = RBF + 1  # rbf channels + bias row
CHUNK = 128  # edges per scatter matmul (contraction dim)
GROUP_CHUNKS = 4
GROUP_E = CHUNK * GROUP_CHUNKS  # 512 edges per elementwise group
DMA_GROUPS = 4  # groups per x DMA (2048 edges, 1 MiB)
DMA_E = GROUP_E * DMA_GROUPS
BF16 = mybir.dt.bfloat16
F32 = mybir.dt.float32
NP_BF16 = mybir.dt.np(BF16)

PACK_FILTER = True  # tile_position row-packing of the 4 K=17 filter matmuls


def _host_prep(x, rbf, num_atoms, edge_index_0, w_rbf, b_rbf):
    """Sort/shard/pad on host with balanced atom binning.

    Atoms are assigned to N_CORES*NT bins (max P atoms each) by greedy LPT on
    edge count, so every bin has nearly equal edges -> minimal padding. Bin b
    maps to core b // NT, atom-tile b % NT, and an atom's one-hot column is
    its position within the bin. Returns the atom->(bin,pos) maps for output
    reassembly.
    """
    import heapq

    n_local = num_atoms // N_CORES
    assert num_atoms % N_CORES == 0
    NT = (n_local + P - 1) // P  # atom tiles per core
    NBINS = N_CORES * NT

    idx = np.asarray(edge_index_0).astype(np.int64)
    counts = np.bincount(idx, minlength=num_atoms)

    # LPT: biggest atoms first into the least-loaded non-full bin
    bin_of_atom = np.empty(num_atoms, dtype=np.int64)
    pos_of_atom = np.empty(num_atoms, dtype=np.int64)
    bin_fill = np.zeros(NBINS, dtype=np.int64)
    heap = [(0, b) for b in range(NBINS)]
    heapq.heapify(heap)
    atom_order = np.argsort(-counts, kind="stable")
    spill = []
    for a in atom_order:
        while True:
            s, b = heapq.heappop(heap)
            if bin_fill[b] < P:
                break
            spill.append((s, b))
        bin_of_atom[a] = b
        pos_of_atom[a] = bin_fill[b]
        bin_fill[b] += 1
        heapq.heappush(heap, (s + int(counts[a]), b))
        for item in spill:
            heapq.heappush(heap, item)
        spill.clear()

    bin_of_edge = bin_of_atom[idx]
    order_all = np.argsort(bin_of_edge, kind="stable")
    bin_counts = np.bincount(bin_of_edge, minlength=NBINS)
    bin_start = np.concatenate([[0], np.cumsum(bin_counts)])

    E_TILE = int(-(-bin_counts.max() // CHUNK) * CHUNK)
    E_PAD = NT * E_TILE  # per-core consumed edge slots
    G = E_PAD // GROUP_E
    assert E_PAD % GROUP_E == 0
    NCHUNK = E_PAD // CHUNK
    CPT = E_TILE // CHUNK  # chunks per atom tile
    D = -(-G // DMA_GROUPS)  # x DMA count (last may be partly consumed)
    E_XG = D * DMA_E

    per_core = []
    for c in range(N_CORES):
        xs = np.zeros((E_XG, C), dtype=np.float32)
        rbf_pad = np.zeros((E_PAD, KF), dtype=np.float32)
        li = np.full((E_PAD,), -1.0, dtype=np.float32)
        for t in range(NT):
            b = c * NT + t
            order = order_all[bin_start[b]:bin_start[b + 1]]
            n = len(order)
            s = t * E_TILE
            xs[s:s + n] = x[order]
            rbf_pad[s:s + n, :RBF] = rbf[order]
            rbf_pad[s:s + n, RBF] = 1.0
            li[s:s + n] = pos_of_atom[idx[order]].astype(np.float32)

        # x: [D, (4 dma-groups, 4 chunks), 128, C] -> [D*128, 16*C]
        # with PACK_FILTER, chunks within a group are stored in the psum
        # evacuation order [0, 2, 1, 3]
        xs4 = xs.reshape(D, DMA_GROUPS, GROUP_CHUNKS, P, C)
        if PACK_FILTER:
            xs4 = xs4[:, :, [0, 2, 1, 3]]
        xg = (
            xs4.reshape(D, DMA_GROUPS * GROUP_CHUNKS, P, C)
            .transpose(0, 2, 1, 3)
            .reshape(D * P, DMA_GROUPS * GROUP_CHUNKS * C)
            .astype(NP_BF16)
        )
        if PACK_FILTER:
            # rbfT packed for 2-row-group tiling: chunk (g,q) on partitions
            # [KF*(q%2), +KF) of the compact array, cols [g*256+(q//2)*128)
            arr = rbf_pad.reshape(G, GROUP_CHUNKS, P, KF)
            rbfT = np.zeros((2 * KF, G, 2, P), dtype=np.float32)
            for q in range(GROUP_CHUNKS):
                rbfT[KF * (q % 2):KF * (q % 2) + KF, :, q // 2, :] = (
                    arr[:, q].transpose(2, 0, 1)
                )
            rbfT = rbfT.reshape(2 * KF, G * 2 * P).astype(NP_BF16)
        else:
            rbfT = np.ascontiguousarray(rbf_pad.T).astype(NP_BF16)
        liT = np.ascontiguousarray(li.reshape(NCHUNK, P).T)  # f32 [P, NCHUNK]
        per_core.append({"xg": xg, "rbfT": rbfT, "liT": liT})

    wrbfT = np.concatenate(
        [w_rbf.T.astype(np.float32), b_rbf[None].astype(np.float32)], axis=0
    )  # [KF, C]
    if PACK_FILTER:
        w4 = np.zeros((P, C), dtype=np.float32)
        for rg in (0, 32):
            w4[rg:rg + KF] = wrbfT
        wrbfT = w4
    shared = {
        "wrbfT": wrbfT.astype(NP_BF16),
        "iota": np.tile(np.arange(P, dtype=np.float32), (P, 1)).astype(NP_BF16),
    }
    dims = dict(NT=NT, A_PAD=NT * P, E_TILE=E_TILE, G=G, E_PAD=E_PAD,
                NCHUNK=NCHUNK, CPT=CPT, n_local=n_local, D=D,
                bin_of_atom=bin_of_atom, pos_of_atom=pos_of_atom)
    return per_core, shared, dims


def _mlp_weights(w1, b1, w2, b2, w3, b3):
    def wT_blocks(w):  # w [out, in] -> lhsT blocks [P, in//P, out]
        wt = w.T.astype(np.float32)  # [in, out]
        i_dim, o_dim = wt.shape
        return np.ascontiguousarray(
            wt.reshape(i_dim // P, P, o_dim).transpose(1, 0, 2)
        ).astype(NP_BF16)

    def b_blocks(b):  # [out] -> [P, out//P]
        return np.ascontiguousarray(b.astype(np.float32).reshape(-1, P).T)

    return {
        "w1T": wT_blocks(w1),
        "w2T": wT_blocks(w2),
        "w3T": wT_blocks(w3),  # [P, 2, 1]
        "b1": b_blocks(b1),
        "b2": b_blocks(b2),
    }, float(np.asarray(b3).reshape(-1)[0])


def _build_bass(dims, b3val):
    NT = dims["NT"]
    A_PAD = dims["A_PAD"]
    G = dims["G"]
    E_PAD = dims["E_PAD"]
    NCHUNK = dims["NCHUNK"]
    CPT = dims["CPT"]  # chunks per atom tile
    D = dims["D"]
    A_PAD_CONST = A_PAD
    GC = GROUP_CHUNKS * C  # elementwise group width (1024)
    XC = DMA_GROUPS * GC  # x DMA tile width (4096)

    nc = bacc.Bacc("TRN2", target_bir_lowering=False, debug=False,
                   num_devices=N_CORES)
    xg_d = nc.dram_tensor("xg", [D * P, XC], BF16, kind="ExternalInput")
    rbf_shape = [2 * KF, G * 2 * P] if PACK_FILTER else [KF, E_PAD]
    rbfT_d = nc.dram_tensor("rbfT", rbf_shape, BF16, kind="ExternalInput")
    liT_d = nc.dram_tensor("liT", [P, NCHUNK], F32, kind="ExternalInput")
    wrbf_shape = [P, C] if PACK_FILTER else [KF, C]
    wrbfT_d = nc.dram_tensor("wrbfT", wrbf_shape, BF16, kind="ExternalInput")
    iota_d = nc.dram_tensor("iota", [P, P], BF16, kind="ExternalInput")
    w1T_d = nc.dram_tensor("w1T", [P, 2, C], BF16, kind="ExternalInput")
    w2T_d = nc.dram_tensor("w2T", [P, 2, C], BF16, kind="ExternalInput")
    w3T_d = nc.dram_tensor("w3T", [P, 2, 1], BF16, kind="ExternalInput")
    b1_d = nc.dram_tensor("b1", [P, 2], F32, kind="ExternalInput")
    b2_d = nc.dram_tensor("b2", [P, 2], F32, kind="ExternalInput")
    y_d = nc.dram_tensor("y", [1, A_PAD], F32, kind="ExternalOutput")

    with tile.TileContext(nc) as tc:
        with (
            tc.tile_pool(name="const", bufs=1) as constp,
            tc.tile_pool(name="pers", bufs=1) as pers,
            tc.tile_pool(name="xt", bufs=4) as xtp,
            tc.tile_pool(name="fsb", bufs=5) as fsbp,
            tc.tile_pool(name="msg", bufs=5) as msgp,
            tc.tile_pool(name="oh", bufs=24) as ohp,
            tc.tile_pool(name="fps", bufs=2, space="PSUM") as fpsp,
            tc.tile_pool(name="sps", bufs=2, space="PSUM") as spsp,
            tc.tile_pool(name="tps", bufs=1, space="PSUM") as tpsp,
            tc.tile_pool(name="mps", bufs=1, space="PSUM") as mpsp,
        ):
            # --- constants: order matters for startup latency ---
            iota_sb = constp.tile([P, P], BF16)
            nc.sync.dma_start(iota_sb[:], iota_d[:])
            wrbfT_sb = constp.tile(wrbf_shape, BF16)
            nc.sync.dma_start(wrbfT_sb[:], wrbfT_d[:])
            sb_rbf_shape = [49, rbf_shape[1]] if PACK_FILTER else rbf_shape
            rbfT_sb = constp.tile(sb_rbf_shape, BF16)
            head = min(rbf_shape[1], 2 * (rbf_shape[1] // NT))

            def dma_rbf(c0, c1):
                if PACK_FILTER:
                    nc.sync.dma_start(rbfT_sb[0:KF, c0:c1],
                                      rbfT_d[0:KF, c0:c1])
                    nc.sync.dma_start(rbfT_sb[32:32 + KF, c0:c1],
                                      rbfT_d[KF:2 * KF, c0:c1])
                else:
                    nc.sync.dma_start(rbfT_sb[:, c0:c1], rbfT_d[:, c0:c1])

            dma_rbf(0, head)

            xts = {}
            fpss = {}
            spsums = {}
            mlp_points = {}  # last tile index -> list of n0 chunks ready
            for n0 in range(0, A_PAD_CONST, 512):
                nsz = min(512, A_PAD_CONST - n0)
                t_req = (n0 + nsz - 1) // P
                mlp_points.setdefault(t_req, []).append(n0)

            def emit_dma(d):
                xt = xtp.tile([P, XC], BF16, name="xt", tag="xt")
                nc.sync.dma_start(xt[:], xg_d[d * P:(d + 1) * P, :])
                xts[d] = xt

            def emit_filter(g):
                if PACK_FILTER:
                    fps = fpsp.tile([P, 2, 512], F32, name="fps", tag="fps")
                else:
                    fps = fpsp.tile([P, GC], F32, name="fps", tag="fps")
                for q in range(GROUP_CHUNKS):
                    ch = g * GROUP_CHUNKS + q
                    if PACK_FILTER:
                        rg = 32 * (q % 2)
                        nc.tensor.matmul(
                            fps[:, q % 2, (q // 2) * C:(q // 2 + 1) * C],
                            lhsT=rbfT_sb[rg:rg + KF,
                                         g * 2 * P + (q // 2) * P:
                                         g * 2 * P + (q // 2 + 1) * P],
                            rhs=wrbfT_sb[rg:rg + KF, :],
                            start=True,
                            stop=True,
                            tile_position=(rg, 0),
                        )
                    else:
                        nc.tensor.matmul(
                            fps[:, q * C:(q + 1) * C],
                            lhsT=rbfT_sb[:, ch * P:(ch + 1) * P],
                            rhs=wrbfT_sb[:],
                            start=True,
                            stop=True,
                        )
                fpss[g] = fps

            def emit_consume(g):
                fps = fpss.pop(g)
                xt = xts[g // DMA_GROUPS]
                g2 = g % DMA_GROUPS
                fsb = fsbp.tile([P, GC], BF16, name="fsb", tag="fsb")
                fps_ap = fps[:] if PACK_FILTER else fps[:]
                if g % 5 == 2:
                    nc.vector.tensor_copy(fsb[:], fps_ap)
                else:
                    nc.scalar.activation(
                        fsb[:], fps_ap, mybir.ActivationFunctionType.Copy,
                    )
                msg = msgp.tile([P, GC], BF16, name="msg", tag="msg")
                nc.vector.tensor_tensor(
                    out=msg[:], in0=fsb[:],
                    in1=xt[:, g2 * GC:(g2 + 1) * GC],
                    op=mybir.AluOpType.mult,
                )
                for q in range(GROUP_CHUNKS):
                    ch = g * GROUP_CHUNKS + q
                    t, ct = divmod(ch, CPT)
                    if ct == 0:
                        spsums[t] = spsp.tile([P, C], F32, name="spsum",
                                              tag="sps")
                    oh = ohp.tile([P, P], BF16, name="oh", tag="oh")
                    oh_eng = nc.vector if ch % 8 == 7 else nc.gpsimd
                    oh_eng.tensor_scalar(
                        oh[:], iota_sb[:], liT_sb[:, ch:ch + 1], None,
                        mybir.AluOpType.is_equal,
                    )
                    pq = (2 * (q % 2) + q // 2) if PACK_FILTER else q
                    nc.tensor.matmul(
                        spsums[t][:],
                        lhsT=oh[:],
                        rhs=msg[:, pq * C:(pq + 1) * C],
                        start=(ct == 0),
                        stop=(ct == CPT - 1),
                    )
                    if ct == CPT - 1:
                        emit_tile_end(t)
                        for n0 in mlp_points.get(t, []):
                            emit_mlp_chunk(n0)

            def emit_tile_end(t):
                nc.any.tensor_copy(h0_all[:, t * C:(t + 1) * C],
                                   spsums.pop(t)[:])
                for k in range(2):
                    tps = tpsp.tile([P, P], BF16, name="tps", tag="tps")
                    nc.tensor.transpose(
                        tps[:],
                        h0_all[:, t * C + k * P: t * C + (k + 1) * P],
                        ident_sb[:],
                    )
                    nc.any.tensor_copy(hT[:, k, t * P:(t + 1) * P], tps[:])

            def emit_mlp_chunk(n0):
                nsz = min(512, A_PAD - n0)

                def layer(src_t, dst, wsb, bsb):
                    mp = mpsp.tile([P, 512], F32, name="mp", tag="mp")
                    for k in range(2):
                        nc.tensor.matmul(
                            mp[:, :nsz],
                            lhsT=wsb[:, k, :] if wsb is w3T_sb
                            else wsb[:, k, 0:P],
                            rhs=src_t[:, k, n0:n0 + nsz],
                            start=(k == 0),
                            stop=(k == 1),
                        )
                    return mp

                for m in range(2):
                    mp = mpsp.tile([P, 512], F32, name="mp", tag="mp")
                    for k in range(2):
                        nc.tensor.matmul(
                            mp[:, :nsz],
                            lhsT=w1T_sb[:, k, m * P:(m + 1) * P],
                            rhs=hT[:, k, n0:n0 + nsz],
                            start=(k == 0), stop=(k == 1),
                        )
                    nc.scalar.activation(
                        h1T[:, m, n0:n0 + nsz], mp[:, :nsz],
                        mybir.ActivationFunctionType.Silu,
                        bias=b1_sb[:, m:m + 1],
                    )
                for m in range(2):
                    mp = mpsp.tile([P, 512], F32, name="mp", tag="mp")
                    for k in range(2):
                        nc.tensor.matmul(
                            mp[:, :nsz],
                            lhsT=w2T_sb[:, k, m * P:(m + 1) * P],
                            rhs=h1T[:, k, n0:n0 + nsz],
                            start=(k == 0), stop=(k == 1),
                        )
                    nc.scalar.activation(
                        h2T[:, m, n0:n0 + nsz], mp[:, :nsz],
                        mybir.ActivationFunctionType.Silu,
                        bias=b2_sb[:, m:m + 1],
                    )
                mp = mpsp.tile([P, 512], F32, name="mp", tag="mp")
                for k in range(2):
                    nc.tensor.matmul(
                        mp[:1, :nsz],
                        lhsT=w3T_sb[:, k, :],
                        rhs=h2T[:, k, n0:n0 + nsz],
                        start=(k == 0), stop=(k == 1),
                    )
                nc.scalar.activation(
                    y_sb[:, n0:n0 + nsz], mp[:1, :nsz],
                    mybir.ActivationFunctionType.Copy, bias=b3val,
                )

            # --- pipelined emission ---
            emit_dma(0)
            emit_filter(0)

            # remaining constants (needed later; after the first x tile)
            liT_sb = constp.tile([P, NCHUNK], F32)
            nc.sync.dma_start(liT_sb[:], liT_d[:])
            if head < rbf_shape[1]:
                dma_rbf(head, rbf_shape[1])
            w1T_sb = constp.tile([P, 2, C], BF16)
            nc.sync.dma_start(w1T_sb[:], w1T_d[:])
            w2T_sb = constp.tile([P, 2, C], BF16)
            nc.sync.dma_start(w2T_sb[:], w2T_d[:])
            w3T_sb = constp.tile([P, 2, 1], BF16)
            nc.sync.dma_start(w3T_sb[:], w3T_d[:])
            b1_sb = constp.tile([P, 2], F32)
            nc.sync.dma_start(b1_sb[:], b1_d[:])
            b2_sb = constp.tile([P, 2], F32)
            nc.sync.dma_start(b2_sb[:], b2_d[:])
            ident_sb = constp.tile([P, P], BF16)
            make_identity(nc, ident_sb[:])

            h0_all = pers.tile([P, NT * C], BF16)
            hT = pers.tile([P, 2, A_PAD], BF16)
            h1T = pers.tile([P, 2, A_PAD], BF16)
            h2T = pers.tile([P, 2, A_PAD], BF16)
            y_sb = pers.tile([1, A_PAD], F32)

            for g in range(G):
                if (g + 1) % DMA_GROUPS == 0 and g + 1 < G:
                    emit_dma((g + 1) // DMA_GROUPS)
                if g + 1 < G:
                    emit_filter(g + 1)
                emit_consume(g)
            nc.sync.dma_start(y_d[:], y_sb[:])

    nc.compile()
    return nc


def _prepare(x, rbf, num_atoms, edge_index_0, w_rbf, b_rbf, w1, b1, w2, b2, w3, b3):
    x = np.asarray(x, dtype=np.float32)
    rbf = np.asarray(rbf, dtype=np.float32)
    num_atoms = int(num_atoms)
    per_core, shared, dims = _host_prep(x, rbf, num_atoms, edge_index_0,
                                        np.asarray(w_rbf, np.float32),
                                        np.asarray(b_rbf, np.float32))
    mlp, b3val = _mlp_weights(
        np.asarray(w1, np.float32), np.asarray(b1, np.float32),
        np.asarray(w2, np.float32), np.asarray(b2, np.float32),
        np.asarray(w3, np.float32), np.asarray(b3, np.float32))
    nc = _build_bass(dims, b3val)
    in_maps = [{**pc, **shared, **mlp} for pc in per_core]
    return nc, in_maps, dims


def assemble_output(res_y, dims, num_atoms):
    """res_y: list of per-core [1, A_PAD] arrays -> [num_atoms, 1]."""
    NT = dims["NT"]
    ys = np.stack([np.asarray(y)[0] for y in res_y])  # [N_CORES, A_PAD]
    b = dims["bin_of_atom"]
    out = ys[b // NT, (b % NT) * P + dims["pos_of_atom"]]
    return out.reshape(num_atoms, 1).astype(np.float32)


def kernel(**inputs) -> np.ndarray:
    num_atoms = int(inputs["num_atoms"])
    nc, in_maps, dims = _prepare(**inputs)
    res = run_bass_kernel_spmd(nc, in_maps, core_ids=list(range(N_CORES)))
    return assemble_output([r["y"] for r in res.results], dims, num_atoms)


# revision 23
# speedup vs baseline: 562.6753x; 562.6753x over previous
"""Trainium2 Bass kernel for AtomWise GNN message passing.

reference:
    rbf_filter = rbf @ w_rbf.T + b_rbf        # [E, C]
    msg = rbf_filter * x                      # [E, C]
    out = segment_sum(msg, edge_index_0, N)   # [N, C]
    out = silu(out @ w1.T + b1); out = silu(out @ w2.T + b2); out = out @ w3.T + b3

Strategy (8 NeuronCores, no collectives):
  - Host: stable-sort edges by destination atom; shard ATOMS (N/8 per core) so
    each core owns all edges of its atom range.  Within a core, atoms are
    processed in 128-atom tiles; each tile's edge list is padded to a global
    E_TILE so every core runs the identical SPMD program.
  - Device (per core, per 512-edge group):
      PE:  filter = rbf_chunk(K=17, bias row folded) @ w_rbfT -> PSUM
           (4 chunks packed into row-groups via tile_position, running
            concurrently on 32-row strips of the PE array)
      ACT: evacuate filter PSUM -> SBUF bf16
      DVE: msg = filter * x  (bf16 2x mode)
      DVE: one-hot[e, a] = (iota_row == li[e])  (tensor_scalar is_equal, 4x)
      PE:  atom_psum[a, c] += one-hot.T @ msg   (scatter-add as matmul)
    Then per-atom-tile PSUM -> SBUF, PE transposes to [C, atoms] layout and a
    3-layer MLP (bf16 matmuls, f32 accumulate) runs on-chip; output [1, atoms].
"""

import numpy as np

import concourse.bacc as bacc
import concourse.mybir as mybir
import concourse.tile as tile
from concourse.bass_utils import run_bass_kernel_spmd
from concourse.masks import make_identity

N_CORES = 8
P = 128
C = 256
RBF = 16
KF = RBF + 1  # rbf channels + bias row
CHUNK = 128  # edges per scatter matmul (contraction dim)
GROUP_CHUNKS = 4
GROUP_E = CHUNK * GROUP_CHUNKS  # 512 edges per elementwise group
DMA_GROUPS = 4  # groups per x DMA (2048 edges, 1 MiB)
DMA_E = GROUP_E * DMA_GROUPS
BF16 = mybir.dt.bfloat16
F32 = mybir.dt.float32
NP_BF16 = mybir.dt.np(BF16)

PACK_FILTER = True  # tile_position row-packing of the 4 K=17 filter matmuls


def _host_prep(x, rbf, num_atoms, edge_index_0, w_rbf, b_rbf):
    """Sort/shard/pad on host with balanced atom binning.

    Atoms are assigned to N_CORES*NT bins (max P atoms each) by greedy LPT on
    edge count, so every bin has nearly equal edges -> minimal padding. Bin b
    maps to core b // NT, atom-tile b % NT, and an atom's one-hot column is
    its position within the bin. Returns the atom->(bin,pos) maps for output
    reassembly.
    """
    import heapq

    n_local = num_atoms // N_CORES
    assert num_atoms % N_CORES == 0
    NT = (n_local + P - 1) // P  # atom tiles per core
    NBINS = N_CORES * NT

    idx = np.asarray(edge_index_0).astype(np.int64)
    counts = np.bincount(idx, minlength=num_atoms)

    # LPT: biggest atoms first into the least-loaded non-full bin
    bin_of_atom = np.empty(num_atoms, dtype=np.int64)
    pos_of_atom = np.empty(num_atoms, dtype=np.int64)
    bin_fill = np.zeros(NBINS, dtype=np.int64)
    heap = [(0, b) for b in range(NBINS)]
    heapq.heapify(heap)
    atom_order = np.argsort(-counts, kind="stable")
    spill = []
    for a in atom_order:
        while True:
            s, b = heapq.heappop(heap)
            if bin_fill[b] < P:
                break
            spill.append((s, b))
        bin_of_atom[a] = b
        pos_of_atom[a] = bin_fill[b]
        bin_fill[b] += 1
        heapq.heappush(heap, (s + int(counts[a]), b))
        for item in spill:
            heapq.heappush(heap, item)
        spill.clear()

    bin_of_edge = bin_of_atom[idx]
    order_all = np.argsort(bin_of_edge, kind="stable")
    bin_counts = np.bincount(bin_of_edge, minlength=NBINS)
    bin_start = np.concatenate([[0], np.cumsum(bin_counts)])

    E_TILE = int(-(-bin_counts.max() // CHUNK) * CHUNK)
    while (NT * E_TILE) % GROUP_E != 0:
        E_TILE += CHUNK
    E_PAD = NT * E_TILE  # per-core consumed edge slots
    G = E_PAD // GROUP_E
    NCHUNK = E_PAD // CHUNK
    CPT = E_TILE // CHUNK  # chunks per atom tile
    D = -(-G // DMA_GROUPS)  # x DMA count (last may be partly consumed)
    E_XG = D * DMA_E

    per_core = []
    for c in range(N_CORES):
        xs = np.zeros((E_XG, C), dtype=np.float32)
        rbf_pad = np.zeros((E_PAD, KF), dtype=np.float32)
        li = np.full((E_PAD,), -1.0, dtype=np.float32)
        for t in range(NT):
            b = c * NT + t
            order = order_all[bin_start[b]:bin_start[b + 1]]
            n = len(order)
            s = t * E_TILE
            xs[s:s + n] = x[order]
            rbf_pad[s:s + n, :RBF] = rbf[order]
            rbf_pad[s:s + n, RBF] = 1.0
            li[s:s + n] = pos_of_atom[idx[order]].astype(np.float32)

        # x: [D, (4 dma-groups, 4 chunks), 128, C] -> [D*128, 16*C]
        # with PACK_FILTER, chunks within a group are stored in the psum
        # evacuation order [0, 2, 1, 3]
        xs4 = xs.reshape(D, DMA_GROUPS, GROUP_CHUNKS, P, C)
        if PACK_FILTER:
            xs4 = xs4[:, :, [0, 2, 1, 3]]
        xg = (
            xs4.reshape(D, DMA_GROUPS * GROUP_CHUNKS, P, C)
            .transpose(0, 2, 1, 3)
            .reshape(D * P, DMA_GROUPS * GROUP_CHUNKS * C)
            .astype(NP_BF16)
        )
        if PACK_FILTER:
            # rbfT packed for 2-row-group tiling: chunk (g,q) on partitions
            # [KF*(q%2), +KF) of the compact array, cols [g*256+(q//2)*128)
            arr = rbf_pad.reshape(G, GROUP_CHUNKS, P, KF)
            rbfT = np.zeros((2 * KF, G, 2, P), dtype=np.float32)
            for q in range(GROUP_CHUNKS):
                rbfT[KF * (q % 2):KF * (q % 2) + KF, :, q // 2, :] = (
                    arr[:, q].transpose(2, 0, 1)
                )
            rbfT = rbfT.reshape(2 * KF, G * 2 * P).astype(NP_BF16)
        else:
            rbfT = np.ascontiguousarray(rbf_pad.T).astype(NP_BF16)
        liT = np.ascontiguousarray(li.reshape(NCHUNK, P).T)  # f32 [P, NCHUNK]
        per_core.append({"xg": xg, "rbfT": rbfT, "_liT": liT})

    wrbfT = np.concatenate(
        [w_rbf.T.astype(np.float32), b_rbf[None].astype(np.float32)], axis=0
    )  # [KF, C]
    if PACK_FILTER:
        w4 = np.zeros((P, C), dtype=np.float32)
        for rg in (0, 32):
            w4[rg:rg + KF] = wrbfT
        wrbfT = w4
    iota = np.tile(np.arange(P, dtype=np.float32), (P, 1))
    # bf16 bundle: [iota(128) | wrbfT(C)]
    pb = np.concatenate([iota, np.zeros((P, C), np.float32)], axis=1)
    pb[:wrbfT.shape[0], P:P + C] = wrbfT
    shared = {"params_bf": pb.astype(NP_BF16)}
    dims = dict(NT=NT, A_PAD=NT * P, E_TILE=E_TILE, G=G, E_PAD=E_PAD,
                NCHUNK=NCHUNK, CPT=CPT, n_local=n_local, D=D,
                bin_of_atom=bin_of_atom, pos_of_atom=pos_of_atom)
    return per_core, shared, dims


def _mlp_weights(w1, b1, w2, b2, w3, b3):
    def wT_blocks(w):  # w [out, in] -> lhsT blocks [P, in//P, out]
        wt = w.T.astype(np.float32)  # [in, out]
        i_dim, o_dim = wt.shape
        return np.ascontiguousarray(
            wt.reshape(i_dim // P, P, o_dim).transpose(1, 0, 2)
        ).astype(NP_BF16).astype(np.float32)

    def b_blocks(b):  # [out] -> [P, out//P]
        return np.ascontiguousarray(b.astype(np.float32).reshape(-1, P).T)

    wb = np.concatenate([
        wT_blocks(w1).reshape(P, 2 * C).astype(np.float32),
        wT_blocks(w2).reshape(P, 2 * C).astype(np.float32),
        wT_blocks(w3).reshape(P, 2).astype(np.float32),
    ], axis=1)  # [P, 4C+2] -> appended to params_bf
    fb = np.concatenate([b_blocks(b1), b_blocks(b2)], axis=1)  # [P, 4]
    return wb, fb, float(np.asarray(b3).reshape(-1)[0])


def _build_bass(dims, b3val):
    NT = dims["NT"]
    A_PAD = dims["A_PAD"]
    G = dims["G"]
    E_PAD = dims["E_PAD"]
    NCHUNK = dims["NCHUNK"]
    CPT = dims["CPT"]  # chunks per atom tile
    D = dims["D"]
    A_PAD_CONST = A_PAD
    GC = GROUP_CHUNKS * C  # elementwise group width (1024)
    XC = DMA_GROUPS * GC  # x DMA tile width (4096)

    nc = bacc.Bacc("TRN2", target_bir_lowering=False, debug=False,
                   num_devices=N_CORES)
    xg_d = nc.dram_tensor("xg", [D * P, XC], BF16, kind="ExternalInput")
    rbf_shape = [2 * KF, G * 2 * P] if PACK_FILTER else [KF, E_PAD]
    rbfT_d = nc.dram_tensor("rbfT", rbf_shape, BF16, kind="ExternalInput")
    PBW = P + C + 2 * (2 * C) + 2  # iota | wrbfT | w1T | w2T | w3T
    PFW = NCHUNK + 4  # liT | b1 | b2
    pbf_d = nc.dram_tensor("params_bf", [P, PBW], BF16, kind="ExternalInput")
    pf_d = nc.dram_tensor("params_f32", [P, PFW], F32, kind="ExternalInput")
    y_d = nc.dram_tensor("y", [1, A_PAD], F32, kind="ExternalOutput")

    with tile.TileContext(nc) as tc:
        with (
            tc.tile_pool(name="const", bufs=1) as constp,
            tc.tile_pool(name="pers", bufs=1) as pers,
            tc.tile_pool(name="xt", bufs=4) as xtp,
            tc.tile_pool(name="fsb", bufs=5) as fsbp,
            tc.tile_pool(name="msg", bufs=5) as msgp,
            tc.tile_pool(name="oh", bufs=24) as ohp,
            tc.tile_pool(name="fps", bufs=2, space="PSUM") as fpsp,
            tc.tile_pool(name="sps", bufs=2, space="PSUM") as spsp,
            tc.tile_pool(name="tps", bufs=1, space="PSUM") as tpsp,
            tc.tile_pool(name="mps", bufs=1, space="PSUM") as mpsp,
        ):
            # --- constants: one bundled DMA each for bf16/f32 params ---
            pbf_sb = constp.tile([P, PBW], BF16)
            nc.sync.dma_start(pbf_sb[:], pbf_d[:])
            pf_sb = constp.tile([P, PFW], F32)
            nc.sync.dma_start(pf_sb[:], pf_d[:])
            iota_sb = pbf_sb[:, 0:P]
            wrbfT_sb = pbf_sb[:, P:P + C]
            w1T_sb = pbf_sb[:, P + C:P + C + 2 * C].rearrange(
                "p (k c) -> p k c", k=2)
            w2T_sb = pbf_sb[:, P + 3 * C:P + 5 * C].rearrange(
                "p (k c) -> p k c", k=2)
            w3T_sb = pbf_sb[:, P + 5 * C:P + 5 * C + 2].rearrange(
                "p (k c) -> p k c", k=2)
            liT_sb = pf_sb[:, 0:NCHUNK]
            b1_sb = pf_sb[:, NCHUNK:NCHUNK + 2]
            b2_sb = pf_sb[:, NCHUNK + 2:NCHUNK + 4]
            sb_rbf_shape = [49, rbf_shape[1]] if PACK_FILTER else rbf_shape
            rbfT_sb = constp.tile(sb_rbf_shape, BF16)
            head = min(rbf_shape[1], 2 * (rbf_shape[1] // NT))

            def dma_rbf(c0, c1):
                if PACK_FILTER:
                    nc.sync.dma_start(rbfT_sb[0:KF, c0:c1],
                                      rbfT_d[0:KF, c0:c1])
                    nc.sync.dma_start(rbfT_sb[32:32 + KF, c0:c1],
                                      rbfT_d[KF:2 * KF, c0:c1])
                else:
                    nc.sync.dma_start(rbfT_sb[:, c0:c1], rbfT_d[:, c0:c1])

            dma_rbf(0, head)

            xts = {}
            fpss = {}
            spsums = {}
            mlp_points = {}  # last tile index -> list of n0 chunks ready
            for n0 in range(0, A_PAD_CONST, 512):
                nsz = min(512, A_PAD_CONST - n0)
                t_req = (n0 + nsz - 1) // P
                mlp_points.setdefault(t_req, []).append(n0)

            def emit_dma(d):
                xt = xtp.tile([P, XC], BF16, name="xt", tag="xt")
                nc.sync.dma_start(xt[:], xg_d[d * P:(d + 1) * P, :])
                xts[d] = xt

            def emit_filter(g):
                if PACK_FILTER:
                    fps = fpsp.tile([P, 2, 512], F32, name="fps", tag="fps")
                else:
                    fps = fpsp.tile([P, GC], F32, name="fps", tag="fps")
                for q in range(GROUP_CHUNKS):
                    ch = g * GROUP_CHUNKS + q
                    if PACK_FILTER:
                        rg = 32 * (q % 2)
                        nc.tensor.matmul(
                            fps[:, q % 2, (q // 2) * C:(q // 2 + 1) * C],
                            lhsT=rbfT_sb[rg:rg + KF,
                                         g * 2 * P + (q // 2) * P:
                                         g * 2 * P + (q // 2 + 1) * P],
                            rhs=wrbfT_sb[rg:rg + KF, :],
                            start=True,
                            stop=True,
                            tile_position=(rg, 0),
                        )
                    else:
                        nc.tensor.matmul(
                            fps[:, q * C:(q + 1) * C],
                            lhsT=rbfT_sb[:, ch * P:(ch + 1) * P],
                            rhs=wrbfT_sb[:],
                            start=True,
                            stop=True,
                        )
                fpss[g] = fps

            def emit_consume(g):
                fps = fpss.pop(g)
                xt = xts[g // DMA_GROUPS]
                g2 = g % DMA_GROUPS
                fsb = fsbp.tile([P, GC], BF16, name="fsb", tag="fsb")
                fps_ap = fps[:] if PACK_FILTER else fps[:]
                if g % 5 == 2:
                    nc.vector.tensor_copy(fsb[:], fps_ap)
                else:
                    nc.scalar.activation(
                        fsb[:], fps_ap, mybir.ActivationFunctionType.Copy,
                    )
                msg = msgp.tile([P, GC], BF16, name="msg", tag="msg")
                nc.vector.tensor_tensor(
                    out=msg[:], in0=fsb[:],
                    in1=xt[:, g2 * GC:(g2 + 1) * GC],
                    op=mybir.AluOpType.mult,
                )
                for q in range(GROUP_CHUNKS):
                    ch = g * GROUP_CHUNKS + q
                    t, ct = divmod(ch, CPT)
                    if ct == 0:
                        spsums[t] = spsp.tile([P, C], F32, name="spsum",
                                              tag="sps")
                    oh = ohp.tile([P, P], BF16, name="oh", tag="oh")
                    oh_eng = nc.vector if ch % 8 == 7 else nc.gpsimd
                    oh_eng.tensor_scalar(
                        oh[:], iota_sb[:], liT_sb[:, ch:ch + 1], None,
                        mybir.AluOpType.is_equal,
                    )
                    pq = (2 * (q % 2) + q // 2) if PACK_FILTER else q
                    nc.tensor.matmul(
                        spsums[t][:],
                        lhsT=oh[:],
                        rhs=msg[:, pq * C:(pq + 1) * C],
                        start=(ct == 0),
                        stop=(ct == CPT - 1),
                    )
                    if ct == CPT - 1:
                        emit_tile_end(t)
                        for n0 in mlp_points.get(t, []):
                            emit_mlp_chunk(n0)

            def emit_tile_end(t):
                nc.any.tensor_copy(h0_all[:, t * C:(t + 1) * C],
                                   spsums.pop(t)[:])
                for k in range(2):
                    tps = tpsp.tile([P, P], BF16, name="tps", tag="tps")
                    nc.tensor.transpose(
                        tps[:],
                        h0_all[:, t * C + k * P: t * C + (k + 1) * P],
                        ident_sb[:],
                    )
                    nc.any.tensor_copy(hT[:, k, t * P:(t + 1) * P], tps[:])

            def emit_mlp_chunk(n0):
                nsz = min(512, A_PAD - n0)

                def layer(src_t, dst, wsb, bsb):
                    mp = mpsp.tile([P, 512], F32, name="mp", tag="mp")
                    for k in range(2):
                        nc.tensor.matmul(
                            mp[:, :nsz],
                            lhsT=wsb[:, k, :] if wsb is w3T_sb
                            else wsb[:, k, 0:P],
                            rhs=src_t[:, k, n0:n0 + nsz],
                            start=(k == 0),
                            stop=(k == 1),
                        )
                    return mp

                for m in range(2):
                    mp = mpsp.tile([P, 512], F32, name="mp", tag="mp")
                    for k in range(2):
                        nc.tensor.matmul(
                            mp[:, :nsz],
                            lhsT=w1T_sb[:, k, m * P:(m + 1) * P],
                            rhs=hT[:, k, n0:n0 + nsz],
                            start=(k == 0), stop=(k == 1),
                        )
                    nc.scalar.activation(
                        h1T[:, m, n0:n0 + nsz], mp[:, :nsz],
                        mybir.ActivationFunctionType.Silu,
                        bias=b1_sb[:, m:m + 1],
                    )
                for m in range(2):
                    mp = mpsp.tile([P, 512], F32, name="mp", tag="mp")
                    for k in range(2):
                        nc.tensor.matmul(
                            mp[:, :nsz],
                            lhsT=w2T_sb[:, k, m * P:(m + 1) * P],
                            rhs=h1T[:, k, n0:n0 + nsz],
                            start=(k == 0), stop=(k == 1),
                        )
                    nc.scalar.activation(
                        h2T[:, m, n0:n0 + nsz], mp[:, :nsz],
                        mybir.ActivationFunctionType.Silu,
                        bias=b2_sb[:, m:m + 1],
                    )
                mp = mpsp.tile([P, 512], F32, name="mp", tag="mp")
                for k in range(2):
                    nc.tensor.matmul(
                        mp[:1, :nsz],
                        lhsT=w3T_sb[:, k, :],
                        rhs=h2T[:, k, n0:n0 + nsz],
                        start=(k == 0), stop=(k == 1),
                    )
                nc.scalar.activation(
                    y_sb[:, n0:n0 + nsz], mp[:1, :nsz],
                    mybir.ActivationFunctionType.Copy, bias=b3val,
                )

            # --- pipelined emission ---
            emit_dma(0)
            emit_filter(0)

            # remaining constants (needed later; after the first x tile)
            if head < rbf_shape[1]:
                dma_rbf(head, rbf_shape[1])
            ident_sb = constp.tile([P, P], BF16)
            make_identity(nc, ident_sb[:])

            h0_all = pers.tile([P, NT * C], BF16)
            hT = pers.tile([P, 2, A_PAD], BF16)
            h1T = pers.tile([P, 2, A_PAD], BF16)
            h2T = pers.tile([P, 2, A_PAD], BF16)
            y_sb = pers.tile([1, A_PAD], F32)

            for g in range(G):
                if (g + 1) % DMA_GROUPS == 0 and g + 1 < G:
                    emit_dma((g + 1) // DMA_GROUPS)
                if g + 1 < G:
                    emit_filter(g + 1)
                emit_consume(g)
            nc.sync.dma_start(y_d[:], y_sb[:])

    nc.compile()
    return nc


def _prepare(x, rbf, num_atoms, edge_index_0, w_rbf, b_rbf, w1, b1, w2, b2, w3, b3):
    x = np.asarray(x, dtype=np.float32)
    rbf = np.asarray(rbf, dtype=np.float32)
    num_atoms = int(num_atoms)
    per_core, shared, dims = _host_prep(x, rbf, num_atoms, edge_index_0,
                                        np.asarray(w_rbf, np.float32),
                                        np.asarray(b_rbf, np.float32))
    wb, fb, b3val = _mlp_weights(
        np.asarray(w1, np.float32), np.asarray(b1, np.float32),
        np.asarray(w2, np.float32), np.asarray(b2, np.float32),
        np.asarray(w3, np.float32), np.asarray(b3, np.float32))
    params_bf = np.concatenate(
        [shared["params_bf"].astype(np.float32), wb], axis=1).astype(NP_BF16)
    nc = _build_bass(dims, b3val)
    in_maps = []
    for pc in per_core:
        params_f32 = np.concatenate([pc["_liT"], fb], axis=1).astype(np.float32)
        in_maps.append({"xg": pc["xg"], "rbfT": pc["rbfT"],
                        "params_bf": params_bf, "params_f32": params_f32})
    return nc, in_maps, dims


def assemble_output(res_y, dims, num_atoms):
    """res_y: list of per-core [1, A_PAD] arrays -> [num_atoms, 1]."""
    NT = dims["NT"]
    ys = np.stack([np.asarray(y)[0] for y in res_y])  # [N_CORES, A_PAD]
    b = dims["bin_of_atom"]
    out = ys[b // NT, (b % NT) * P + dims["pos_of_atom"]]
    return out.reshape(num_atoms, 1).astype(np.float32)


def kernel(**inputs) -> np.ndarray:
    num_atoms = int(inputs["num_atoms"])
    nc, in_maps, dims = _prepare(**inputs)
    res = run_bass_kernel_spmd(nc, in_maps, core_ids=list(range(N_CORES)))
    return assemble_output([r["y"] for r in res.results], dims, num_atoms)


# revision 26
# speedup vs baseline: 564.9210x; 1.0040x over previous
"""Trainium2 Bass kernel for AtomWise GNN message passing.

reference:
    rbf_filter = rbf @ w_rbf.T + b_rbf        # [E, C]
    msg = rbf_filter * x                      # [E, C]
    out = segment_sum(msg, edge_index_0, N)   # [N, C]
    out = silu(out @ w1.T + b1); out = silu(out @ w2.T + b2); out = out @ w3.T + b3

Strategy (8 NeuronCores, no collectives):
  - Host: stable-sort edges by destination atom; shard ATOMS (N/8 per core) so
    each core owns all edges of its atom range.  Within a core, atoms are
    processed in 128-atom tiles; each tile's edge list is padded to a global
    E_TILE so every core runs the identical SPMD program.
  - Device (per core, per 512-edge group):
      PE:  filter = rbf_chunk(K=17, bias row folded) @ w_rbfT -> PSUM
           (4 chunks packed into row-groups via tile_position, running
            concurrently on 32-row strips of the PE array)
      ACT: evacuate filter PSUM -> SBUF bf16
      DVE: msg = filter * x  (bf16 2x mode)
      DVE: one-hot[e, a] = (iota_row == li[e])  (tensor_scalar is_equal, 4x)
      PE:  atom_psum[a, c] += one-hot.T @ msg   (scatter-add as matmul)
    Then per-atom-tile PSUM -> SBUF, PE transposes to [C, atoms] layout and a
    3-layer MLP (bf16 matmuls, f32 accumulate) runs on-chip; output [1, atoms].
"""

import numpy as np

import concourse.bacc as bacc
import concourse.mybir as mybir
import concourse.tile as tile
from concourse.bass_utils import run_bass_kernel_spmd
from concourse.masks import make_identity

N_CORES = 8
P = 128
C = 256
RBF = 16
KF = RBF + 1  # rbf channels + bias row
CHUNK = 128  # edges per scatter matmul (contraction dim)
GROUP_CHUNKS = 4
GROUP_E = CHUNK * GROUP_CHUNKS  # 512 edges per elementwise group
DMA_GROUPS = 4  # groups per x DMA (2048 edges, 1 MiB)
DMA_E = GROUP_E * DMA_GROUPS
BF16 = mybir.dt.bfloat16
F32 = mybir.dt.float32
NP_BF16 = mybir.dt.np(BF16)

PACK_FILTER = True  # tile_position row-packing of the 4 K=17 filter matmuls


def _host_prep(x, rbf, num_atoms, edge_index_0, w_rbf, b_rbf):
    """Sort/shard/pad on host with balanced atom binning.

    Atoms are assigned to N_CORES*NT bins (max P atoms each) by greedy LPT on
    edge count, so every bin has nearly equal edges -> minimal padding. Bin b
    maps to core b // NT, atom-tile b % NT, and an atom's one-hot column is
    its position within the bin. Returns the atom->(bin,pos) maps for output
    reassembly.
    """
    import heapq

    n_local = num_atoms // N_CORES
    assert num_atoms % N_CORES == 0
    NT = (n_local + P - 1) // P  # atom tiles per core
    NBINS = N_CORES * NT

    idx = np.asarray(edge_index_0).astype(np.int64)
    counts = np.bincount(idx, minlength=num_atoms)

    # LPT: biggest atoms first into the least-loaded non-full bin
    bin_of_atom = np.empty(num_atoms, dtype=np.int64)
    pos_of_atom = np.empty(num_atoms, dtype=np.int64)
    bin_fill = np.zeros(NBINS, dtype=np.int64)
    heap = [(0, b) for b in range(NBINS)]
    heapq.heapify(heap)
    atom_order = np.argsort(-counts, kind="stable")
    spill = []
    for a in atom_order:
        while True:
            s, b = heapq.heappop(heap)
            if bin_fill[b] < P:
                break
            spill.append((s, b))
        bin_of_atom[a] = b
        pos_of_atom[a] = bin_fill[b]
        bin_fill[b] += 1
        heapq.heappush(heap, (s + int(counts[a]), b))
        for item in spill:
            heapq.heappush(heap, item)
        spill.clear()

    bin_of_edge = bin_of_atom[idx]
    order_all = np.argsort(bin_of_edge, kind="stable")
    bin_counts = np.bincount(bin_of_edge, minlength=NBINS)
    bin_start = np.concatenate([[0], np.cumsum(bin_counts)])

    E_TILE = int(-(-bin_counts.max() // CHUNK) * CHUNK)
    while (NT * E_TILE) % GROUP_E != 0:
        E_TILE += CHUNK
    E_PAD = NT * E_TILE  # per-core consumed edge slots
    G = E_PAD // GROUP_E
    NCHUNK = E_PAD // CHUNK
    CPT = E_TILE // CHUNK  # chunks per atom tile
    D = -(-G // DMA_GROUPS)  # x DMA count (last may be partly consumed)
    E_XG = D * DMA_E

    per_core = []
    for c in range(N_CORES):
        xs = np.zeros((E_XG, C), dtype=np.float32)
        rbf_pad = np.zeros((E_PAD, KF), dtype=np.float32)
        li = np.full((E_PAD,), -1.0, dtype=np.float32)
        for t in range(NT):
            b = c * NT + t
            order = order_all[bin_start[b]:bin_start[b + 1]]
            n = len(order)
            s = t * E_TILE
            xs[s:s + n] = x[order]
            rbf_pad[s:s + n, :RBF] = rbf[order]
            rbf_pad[s:s + n, RBF] = 1.0
            li[s:s + n] = pos_of_atom[idx[order]].astype(np.float32)

        # x: [D, (4 dma-groups, 4 chunks), 128, C] -> [D*128, 16*C]
        # with PACK_FILTER, chunks within a group are stored in the psum
        # evacuation order [0, 2, 1, 3]
        xs4 = xs.reshape(D, DMA_GROUPS, GROUP_CHUNKS, P, C)
        if PACK_FILTER:
            xs4 = xs4[:, :, [0, 2, 1, 3]]
        xg = (
            xs4.reshape(D, DMA_GROUPS * GROUP_CHUNKS, P, C)
            .transpose(0, 2, 1, 3)
            .reshape(D * P, DMA_GROUPS * GROUP_CHUNKS * C)
            .astype(NP_BF16)
        )
        if PACK_FILTER:
            # rbfT packed for 2-row-group tiling: chunk (g,q) on partitions
            # [KF*(q%2), +KF) of the compact array, cols [g*256+(q//2)*128)
            arr = rbf_pad.reshape(G, GROUP_CHUNKS, P, KF)
            rbfT = np.zeros((2 * KF, G, 2, P), dtype=np.float32)
            for q in range(GROUP_CHUNKS):
                rbfT[KF * (q % 2):KF * (q % 2) + KF, :, q // 2, :] = (
                    arr[:, q].transpose(2, 0, 1)
                )
            rbfT = rbfT.reshape(2 * KF, G * 2 * P).astype(NP_BF16)
        else:
            rbfT = np.ascontiguousarray(rbf_pad.T).astype(NP_BF16)
        liT = np.ascontiguousarray(li.reshape(NCHUNK, P).T)  # f32 [P, NCHUNK]
        per_core.append({"xg": xg, "rbfT": rbfT, "_liT": liT})

    wrbfT = np.concatenate(
        [w_rbf.T.astype(np.float32), b_rbf[None].astype(np.float32)], axis=0
    )  # [KF, C]
    if PACK_FILTER:
        w4 = np.zeros((P, C), dtype=np.float32)
        for rg in (0, 32):
            w4[rg:rg + KF] = wrbfT
        wrbfT = w4
    iota = np.tile(np.arange(P, dtype=np.float32), (P, 1))
    # bf16 bundle: [iota(128) | wrbfT(C)]
    pb = np.concatenate([iota, np.zeros((P, C), np.float32)], axis=1)
    pb[:wrbfT.shape[0], P:P + C] = wrbfT
    shared = {"params_bf": pb.astype(NP_BF16)}
    dims = dict(NT=NT, A_PAD=NT * P, E_TILE=E_TILE, G=G, E_PAD=E_PAD,
                NCHUNK=NCHUNK, CPT=CPT, n_local=n_local, D=D,
                bin_of_atom=bin_of_atom, pos_of_atom=pos_of_atom)
    return per_core, shared, dims


def _mlp_weights(w1, b1, w2, b2, w3, b3):
    def wT_blocks(w):  # w [out, in] -> lhsT blocks [P, in//P, out]
        wt = w.T.astype(np.float32)  # [in, out]
        i_dim, o_dim = wt.shape
        return np.ascontiguousarray(
            wt.reshape(i_dim // P, P, o_dim).transpose(1, 0, 2)
        ).astype(NP_BF16).astype(np.float32)

    def b_blocks(b):  # [out] -> [P, out//P]
        return np.ascontiguousarray(b.astype(np.float32).reshape(-1, P).T)

    wb = np.concatenate([
        wT_blocks(w1).reshape(P, 2 * C).astype(np.float32),
        wT_blocks(w2).reshape(P, 2 * C).astype(np.float32),
        wT_blocks(w3).reshape(P, 2).astype(np.float32),
    ], axis=1)  # [P, 4C+2] -> appended to params_bf
    fb = np.concatenate([b_blocks(b1), b_blocks(b2)], axis=1)  # [P, 4]
    return wb, fb, float(np.asarray(b3).reshape(-1)[0])


def _build_bass(dims, b3val):
    NT = dims["NT"]
    A_PAD = dims["A_PAD"]
    G = dims["G"]
    E_PAD = dims["E_PAD"]
    NCHUNK = dims["NCHUNK"]
    CPT = dims["CPT"]  # chunks per atom tile
    D = dims["D"]
    A_PAD_CONST = A_PAD
    GC = GROUP_CHUNKS * C  # elementwise group width (1024)
    XC = DMA_GROUPS * GC  # x DMA tile width (4096)

    nc = bacc.Bacc("TRN2", target_bir_lowering=False, debug=False,
                   num_devices=N_CORES)
    xg_d = nc.dram_tensor("xg", [D * P, XC], BF16, kind="ExternalInput")
    rbf_shape = [2 * KF, G * 2 * P] if PACK_FILTER else [KF, E_PAD]
    rbfT_d = nc.dram_tensor("rbfT", rbf_shape, BF16, kind="ExternalInput")
    PBW = P + C + 2 * (2 * C) + 2  # iota | wrbfT | w1T | w2T | w3T
    PFW = NCHUNK + 4  # liT | b1 | b2
    pbf_d = nc.dram_tensor("params_bf", [P, PBW], BF16, kind="ExternalInput")
    pf_d = nc.dram_tensor("params_f32", [P, PFW], F32, kind="ExternalInput")
    y_d = nc.dram_tensor("y", [1, A_PAD], F32, kind="ExternalOutput")

    with tile.TileContext(nc) as tc:
        with (
            tc.tile_pool(name="const", bufs=1) as constp,
            tc.tile_pool(name="pers", bufs=1) as pers,
            tc.tile_pool(name="xt", bufs=4) as xtp,
            tc.tile_pool(name="fsb", bufs=6) as fsbp,
            tc.tile_pool(name="msg", bufs=6) as msgp,
            tc.tile_pool(name="oh", bufs=32) as ohp,
            tc.tile_pool(name="fps", bufs=2, space="PSUM") as fpsp,
            tc.tile_pool(name="sps", bufs=2, space="PSUM") as spsp,
            tc.tile_pool(name="tps", bufs=1, space="PSUM") as tpsp,
            tc.tile_pool(name="mps", bufs=1, space="PSUM") as mpsp,
        ):
            # --- constants: one bundled DMA each for bf16/f32 params ---
            pbf_sb = constp.tile([P, PBW], BF16)
            nc.sync.dma_start(pbf_sb[:], pbf_d[:])
            pf_sb = constp.tile([P, PFW], F32)
            nc.sync.dma_start(pf_sb[:], pf_d[:])
            iota_sb = pbf_sb[:, 0:P]
            wrbfT_sb = pbf_sb[:, P:P + C]
            w1T_sb = pbf_sb[:, P + C:P + C + 2 * C].rearrange(
                "p (k c) -> p k c", k=2)
            w2T_sb = pbf_sb[:, P + 3 * C:P + 5 * C].rearrange(
                "p (k c) -> p k c", k=2)
            w3T_sb = pbf_sb[:, P + 5 * C:P + 5 * C + 2].rearrange(
                "p (k c) -> p k c", k=2)
            liT_sb = pf_sb[:, 0:NCHUNK]
            b1_sb = pf_sb[:, NCHUNK:NCHUNK + 2]
            b2_sb = pf_sb[:, NCHUNK + 2:NCHUNK + 4]
            sb_rbf_shape = [49, rbf_shape[1]] if PACK_FILTER else rbf_shape
            rbfT_sb = constp.tile(sb_rbf_shape, BF16)
            head = min(rbf_shape[1], 2 * (rbf_shape[1] // NT))

            def dma_rbf(c0, c1):
                if PACK_FILTER:
                    nc.sync.dma_start(rbfT_sb[0:KF, c0:c1],
                                      rbfT_d[0:KF, c0:c1])
                    nc.sync.dma_start(rbfT_sb[32:32 + KF, c0:c1],
                                      rbfT_d[KF:2 * KF, c0:c1])
                else:
                    nc.sync.dma_start(rbfT_sb[:, c0:c1], rbfT_d[:, c0:c1])

            dma_rbf(0, head)

            xts = {}
            fpss = {}
            spsums = {}
            mlp_points = {}  # last tile index -> list of n0 chunks ready
            for n0 in range(0, A_PAD_CONST, 512):
                nsz = min(512, A_PAD_CONST - n0)
                t_req = (n0 + nsz - 1) // P
                mlp_points.setdefault(t_req, []).append(n0)

            def emit_dma(d):
                xt = xtp.tile([P, XC], BF16, name="xt", tag="xt")
                nc.sync.dma_start(xt[:], xg_d[d * P:(d + 1) * P, :])
                xts[d] = xt

            def emit_filter(g):
                if PACK_FILTER:
                    fps = fpsp.tile([P, 2, 512], F32, name="fps", tag="fps")
                else:
                    fps = fpsp.tile([P, GC], F32, name="fps", tag="fps")
                for q in range(GROUP_CHUNKS):
                    ch = g * GROUP_CHUNKS + q
                    if PACK_FILTER:
                        rg = 32 * (q % 2)
                        nc.tensor.matmul(
                            fps[:, q % 2, (q // 2) * C:(q // 2 + 1) * C],
                            lhsT=rbfT_sb[rg:rg + KF,
                                         g * 2 * P + (q // 2) * P:
                                         g * 2 * P + (q // 2 + 1) * P],
                            rhs=wrbfT_sb[rg:rg + KF, :],
                            start=True,
                            stop=True,
                            tile_position=(rg, 0),
                        )
                    else:
                        nc.tensor.matmul(
                            fps[:, q * C:(q + 1) * C],
                            lhsT=rbfT_sb[:, ch * P:(ch + 1) * P],
                            rhs=wrbfT_sb[:],
                            start=True,
                            stop=True,
                        )
                fpss[g] = fps

            def emit_consume(g):
                fps = fpss.pop(g)
                xt = xts[g // DMA_GROUPS]
                g2 = g % DMA_GROUPS
                fsb = fsbp.tile([P, GC], BF16, name="fsb", tag="fsb")
                fps_ap = fps[:] if PACK_FILTER else fps[:]
                if g % 5 == 2:
                    nc.vector.tensor_copy(fsb[:], fps_ap)
                else:
                    nc.scalar.activation(
                        fsb[:], fps_ap, mybir.ActivationFunctionType.Copy,
                    )
                msg = msgp.tile([P, GC], BF16, name="msg", tag="msg")
                nc.vector.tensor_tensor(
                    out=msg[:], in0=fsb[:],
                    in1=xt[:, g2 * GC:(g2 + 1) * GC],
                    op=mybir.AluOpType.mult,
                )
                for q in range(GROUP_CHUNKS):
                    ch = g * GROUP_CHUNKS + q
                    t, ct = divmod(ch, CPT)
                    if ct == 0:
                        spsums[t] = spsp.tile([P, C], F32, name="spsum",
                                              tag="sps")
                    oh = ohp.tile([P, P], BF16, name="oh", tag="oh")
                    oh_eng = nc.vector if ch % 8 == 7 else nc.gpsimd
                    oh_eng.tensor_scalar(
                        oh[:], iota_sb[:], liT_sb[:, ch:ch + 1], None,
                        mybir.AluOpType.is_equal,
                    )
                    pq = (2 * (q % 2) + q // 2) if PACK_FILTER else q
                    nc.tensor.matmul(
                        spsums[t][:],
                        lhsT=oh[:],
                        rhs=msg[:, pq * C:(pq + 1) * C],
                        start=(ct == 0),
                        stop=(ct == CPT - 1),
                    )
                    if ct == CPT - 1:
                        emit_tile_end(t)
                        for n0 in mlp_points.get(t, []):
                            emit_mlp_chunk(n0)

            def emit_tile_end(t):
                nc.any.tensor_copy(h0_all[:, t * C:(t + 1) * C],
                                   spsums.pop(t)[:])
                for k in range(2):
                    tps = tpsp.tile([P, P], BF16, name="tps", tag="tps")
                    nc.tensor.transpose(
                        tps[:],
                        h0_all[:, t * C + k * P: t * C + (k + 1) * P],
                        ident_sb[:],
                    )
                    nc.any.tensor_copy(hT[:, k, t * P:(t + 1) * P], tps[:])

            def emit_mlp_chunk(n0):
                nsz = min(512, A_PAD - n0)

                def layer(src_t, dst, wsb, bsb):
                    mp = mpsp.tile([P, 512], F32, name="mp", tag="mp")
                    for k in range(2):
                        nc.tensor.matmul(
                            mp[:, :nsz],
                            lhsT=wsb[:, k, :] if wsb is w3T_sb
                            else wsb[:, k, 0:P],
                            rhs=src_t[:, k, n0:n0 + nsz],
                            start=(k == 0),
                            stop=(k == 1),
                        )
                    return mp

                for m in range(2):
                    mp = mpsp.tile([P, 512], F32, name="mp", tag="mp")
                    for k in range(2):
                        nc.tensor.matmul(
                            mp[:, :nsz],
                            lhsT=w1T_sb[:, k, m * P:(m + 1) * P],
                            rhs=hT[:, k, n0:n0 + nsz],
                            start=(k == 0), stop=(k == 1),
                        )
                    nc.scalar.activation(
                        h1T[:, m, n0:n0 + nsz], mp[:, :nsz],
                        mybir.ActivationFunctionType.Silu,
                        bias=b1_sb[:, m:m + 1],
                    )
                for m in range(2):
                    mp = mpsp.tile([P, 512], F32, name="mp", tag="mp")
                    for k in range(2):
                        nc.tensor.matmul(
                            mp[:, :nsz],
                            lhsT=w2T_sb[:, k, m * P:(m + 1) * P],
                            rhs=h1T[:, k, n0:n0 + nsz],
                            start=(k == 0), stop=(k == 1),
                        )
                    nc.scalar.activation(
                        h2T[:, m, n0:n0 + nsz], mp[:, :nsz],
                        mybir.ActivationFunctionType.Silu,
                        bias=b2_sb[:, m:m + 1],
                    )
                mp = mpsp.tile([P, 512], F32, name="mp", tag="mp")
                for k in range(2):
                    nc.tensor.matmul(
                        mp[:1, :nsz],
                        lhsT=w3T_sb[:, k, :],
                        rhs=h2T[:, k, n0:n0 + nsz],
                        start=(k == 0), stop=(k == 1),
                    )
                nc.scalar.activation(
                    y_sb[:, n0:n0 + nsz], mp[:1, :nsz],
                    mybir.ActivationFunctionType.Copy, bias=b3val,
                )

            # --- pipelined emission ---
            emit_dma(0)
            emit_filter(0)

            # remaining constants (needed later; after the first x tile)
            if head < rbf_shape[1]:
                dma_rbf(head, rbf_shape[1])
            ident_sb = constp.tile([P, P], BF16)
            make_identity(nc, ident_sb[:])

            h0_all = pers.tile([P, NT * C], BF16)
            hT = pers.tile([P, 2, A_PAD], BF16)
            h1T = pers.tile([P, 2, A_PAD], BF16)
            h2T = pers.tile([P, 2, A_PAD], BF16)
            y_sb = pers.tile([1, A_PAD], F32)

            for g in range(G):
                if (g + 1) % DMA_GROUPS == 0 and g + 1 < G:
                    emit_dma((g + 1) // DMA_GROUPS)
                if g + 1 < G:
                    emit_filter(g + 1)
                emit_consume(g)
            nc.sync.dma_start(y_d[:], y_sb[:])

    nc.compile()
    return nc


def _prepare(x, rbf, num_atoms, edge_index_0, w_rbf, b_rbf, w1, b1, w2, b2, w3, b3):
    x = np.asarray(x, dtype=np.float32)
    rbf = np.asarray(rbf, dtype=np.float32)
    num_atoms = int(num_atoms)
    per_core, shared, dims = _host_prep(x, rbf, num_atoms, edge_index_0,
                                        np.asarray(w_rbf, np.float32),
                                        np.asarray(b_rbf, np.float32))
    wb, fb, b3val = _mlp_weights(
        np.asarray(w1, np.float32), np.asarray(b1, np.float32),
        np.asarray(w2, np.float32), np.asarray(b2, np.float32),
        np.asarray(w3, np.float32), np.asarray(b3, np.float32))
    params_bf = np.concatenate(
        [shared["params_bf"].astype(np.float32), wb], axis=1).astype(NP_BF16)
    nc = _build_bass(dims, b3val)
    in_maps = []
    for pc in per_core:
        params_f32 = np.concatenate([pc["_liT"], fb], axis=1).astype(np.float32)
        in_maps.append({"xg": pc["xg"], "rbfT": pc["rbfT"],
                        "params_bf": params_bf, "params_f32": params_f32})
    return nc, in_maps, dims


def assemble_output(res_y, dims, num_atoms):
    """res_y: list of per-core [1, A_PAD] arrays -> [num_atoms, 1]."""
    NT = dims["NT"]
    ys = np.stack([np.asarray(y)[0] for y in res_y])  # [N_CORES, A_PAD]
    b = dims["bin_of_atom"]
    out = ys[b // NT, (b % NT) * P + dims["pos_of_atom"]]
    return out.reshape(num_atoms, 1).astype(np.float32)


def kernel(**inputs) -> np.ndarray:
    num_atoms = int(inputs["num_atoms"])
    nc, in_maps, dims = _prepare(**inputs)
    res = run_bass_kernel_spmd(nc, in_maps, core_ids=list(range(N_CORES)))
    return assemble_output([r["y"] for r in res.results], dims, num_atoms)


# revision 29
# speedup vs baseline: 566.6894x; 1.0031x over previous
"""Trainium2 Bass kernel for AtomWise GNN message passing.

reference:
    rbf_filter = rbf @ w_rbf.T + b_rbf        # [E, C]
    msg = rbf_filter * x                      # [E, C]
    out = segment_sum(msg, edge_index_0, N)   # [N, C]
    out = silu(out @ w1.T + b1); out = silu(out @ w2.T + b2); out = out @ w3.T + b3

Strategy (8 NeuronCores, no collectives):
  - Host: stable-sort edges by destination atom; shard ATOMS (N/8 per core) so
    each core owns all edges of its atom range.  Within a core, atoms are
    processed in 128-atom tiles; each tile's edge list is padded to a global
    E_TILE so every core runs the identical SPMD program.
  - Device (per core, per 512-edge group):
      PE:  filter = rbf_chunk(K=17, bias row folded) @ w_rbfT -> PSUM
      ACT: evacuate filter PSUM -> SBUF bf16
      DVE: msg = filter * x  (bf16 2x mode)
      DVE: one-hot[e, a] = (iota_row == li[e])  (tensor_scalar is_equal, 4x)
      PE:  atom_psum[a, c] += one-hot.T @ msg   (scatter-add as matmul)
    Then per-atom-tile PSUM -> SBUF, PE transposes to [C, atoms] layout and a
    3-layer MLP (bf16 matmuls, f32 accumulate) runs on-chip; output [1, atoms].
"""

import os as _os

# This kernel executes on the neuron/axon PJRT devices; a JAX_PLATFORMS=cpu
# pin (meant for running jax reference oracles on CPU) would hide them.
if _os.environ.get("JAX_PLATFORMS", "") == "cpu":
    _os.environ.pop("JAX_PLATFORMS")

import numpy as np

import concourse.bacc as bacc
import concourse.mybir as mybir
import concourse.tile as tile
from concourse.bass_utils import run_bass_kernel_spmd
from concourse.masks import make_identity

N_CORES = 8
P = 128
C = 256
RBF = 16
KF = RBF + 1  # rbf channels + bias row
CHUNK = 128  # edges per scatter matmul (contraction dim)
GROUP_CHUNKS = 4
GROUP_E = CHUNK * GROUP_CHUNKS  # 512 edges per elementwise group
DMA_GROUPS = 4  # groups per x DMA (2048 edges, 1 MiB)
DMA_E = GROUP_E * DMA_GROUPS
BF16 = mybir.dt.bfloat16
F32 = mybir.dt.float32
NP_BF16 = mybir.dt.np(BF16)

# tile_position row-packing of the K=17 filter matmuls (2 concurrent row
# groups). Disabled: concurrent row-group matmuls draining into PSUM showed a
# rare NRT_EXEC_UNIT_UNRECOVERABLE device fault when two drains land in the
# same PSUM bank window. The serial path is deterministic and ~10% slower on
# the tensor engine only.
PACK_FILTER = False


def _host_prep(x, rbf, num_atoms, edge_index_0, w_rbf, b_rbf):
    """Sort/shard/pad on host with balanced atom binning.

    Atoms are assigned to N_CORES*NT bins (max P atoms each) by greedy LPT on
    edge count, so every bin has nearly equal edges -> minimal padding. Bin b
    maps to core b // NT, atom-tile b % NT, and an atom's one-hot column is
    its position within the bin. Returns the atom->(bin,pos) maps for output
    reassembly.
    """
    import heapq

    n_local = num_atoms // N_CORES
    assert num_atoms % N_CORES == 0
    NT = (n_local + P - 1) // P  # atom tiles per core
    NBINS = N_CORES * NT

    idx = np.asarray(edge_index_0).astype(np.int64)
    counts = np.bincount(idx, minlength=num_atoms)

    # LPT: biggest atoms first into the least-loaded non-full bin
    bin_of_atom = np.empty(num_atoms, dtype=np.int64)
    pos_of_atom = np.empty(num_atoms, dtype=np.int64)
    bin_fill = np.zeros(NBINS, dtype=np.int64)
    heap = [(0, b) for b in range(NBINS)]
    heapq.heapify(heap)
    atom_order = np.argsort(-counts, kind="stable")
    spill = []
    for a in atom_order:
        while True:
            s, b = heapq.heappop(heap)
            if bin_fill[b] < P:
                break
            spill.append((s, b))
        bin_of_atom[a] = b
        pos_of_atom[a] = bin_fill[b]
        bin_fill[b] += 1
        heapq.heappush(heap, (s + int(counts[a]), b))
        for item in spill:
            heapq.heappush(heap, item)
        spill.clear()

    bin_of_edge = bin_of_atom[idx]
    order_all = np.argsort(bin_of_edge, kind="stable")
    bin_counts = np.bincount(bin_of_edge, minlength=NBINS)
    bin_start = np.concatenate([[0], np.cumsum(bin_counts)])

    E_TILE = int(-(-bin_counts.max() // CHUNK) * CHUNK)
    while (NT * E_TILE) % GROUP_E != 0:
        E_TILE += CHUNK
    E_PAD = NT * E_TILE  # per-core consumed edge slots
    G = E_PAD // GROUP_E
    NCHUNK = E_PAD // CHUNK
    CPT = E_TILE // CHUNK  # chunks per atom tile
    D = -(-G // DMA_GROUPS)  # x DMA count (last may be partly consumed)
    E_XG = D * DMA_E

    per_core = []
    for c in range(N_CORES):
        xs = np.zeros((E_XG, C), dtype=np.float32)
        rbf_pad = np.zeros((E_PAD, KF), dtype=np.float32)
        li = np.full((E_PAD,), -1.0, dtype=np.float32)
        for t in range(NT):
            b = c * NT + t
            order = order_all[bin_start[b]:bin_start[b + 1]]
            n = len(order)
            s = t * E_TILE
            xs[s:s + n] = x[order]
            rbf_pad[s:s + n, :RBF] = rbf[order]
            rbf_pad[s:s + n, RBF] = 1.0
            li[s:s + n] = pos_of_atom[idx[order]].astype(np.float32)

        # x: [D, (4 dma-groups, 4 chunks), 128, C] -> [D*128, 16*C]
        # with PACK_FILTER, chunks within a group are stored in the psum
        # evacuation order [0, 2, 1, 3]
        xs4 = xs.reshape(D, DMA_GROUPS, GROUP_CHUNKS, P, C)
        if PACK_FILTER:
            xs4 = xs4[:, :, [0, 2, 1, 3]]
        xg = (
            xs4.reshape(D, DMA_GROUPS * GROUP_CHUNKS, P, C)
            .transpose(0, 2, 1, 3)
            .reshape(D * P, DMA_GROUPS * GROUP_CHUNKS * C)
            .astype(NP_BF16)
        )
        if PACK_FILTER:
            # rbfT packed for 2-row-group tiling: chunk (g,q) on partitions
            # [KF*(q%2), +KF) of the compact array, cols [g*256+(q//2)*128)
            arr = rbf_pad.reshape(G, GROUP_CHUNKS, P, KF)
            rbfT = np.zeros((2 * KF, G, 2, P), dtype=np.float32)
            for q in range(GROUP_CHUNKS):
                rbfT[KF * (q % 2):KF * (q % 2) + KF, :, q // 2, :] = (
                    arr[:, q].transpose(2, 0, 1)
                )
            rbfT = rbfT.reshape(2 * KF, G * 2 * P).astype(NP_BF16)
        else:
            rbfT = np.ascontiguousarray(rbf_pad.T).astype(NP_BF16)
        liT = np.ascontiguousarray(li.reshape(NCHUNK, P).T)  # f32 [P, NCHUNK]
        per_core.append({"xg": xg, "rbfT": rbfT, "_liT": liT})

    wrbfT = np.concatenate(
        [w_rbf.T.astype(np.float32), b_rbf[None].astype(np.float32)], axis=0
    )  # [KF, C]
    if PACK_FILTER:
        w4 = np.zeros((P, C), dtype=np.float32)
        for rg in (0, 32):
            w4[rg:rg + KF] = wrbfT
        wrbfT = w4
    iota = np.tile(np.arange(P, dtype=np.float32), (P, 1))
    # bf16 bundle: [iota(128) | wrbfT(C)]
    pb = np.concatenate([iota, np.zeros((P, C), np.float32)], axis=1)
    pb[:wrbfT.shape[0], P:P + C] = wrbfT
    shared = {"params_bf": pb.astype(NP_BF16)}
    dims = dict(NT=NT, A_PAD=NT * P, E_TILE=E_TILE, G=G, E_PAD=E_PAD,
                NCHUNK=NCHUNK, CPT=CPT, n_local=n_local, D=D,
                bin_of_atom=bin_of_atom, pos_of_atom=pos_of_atom)
    return per_core, shared, dims


def _mlp_weights(w1, b1, w2, b2, w3, b3):
    def wT_blocks(w):  # w [out, in] -> lhsT blocks [P, in//P, out]
        wt = w.T.astype(np.float32)  # [in, out]
        i_dim, o_dim = wt.shape
        return np.ascontiguousarray(
            wt.reshape(i_dim // P, P, o_dim).transpose(1, 0, 2)
        ).astype(NP_BF16).astype(np.float32)

    def b_blocks(b):  # [out] -> [P, out//P]
        return np.ascontiguousarray(b.astype(np.float32).reshape(-1, P).T)

    wb = np.concatenate([
        wT_blocks(w1).reshape(P, 2 * C).astype(np.float32),
        wT_blocks(w2).reshape(P, 2 * C).astype(np.float32),
        wT_blocks(w3).reshape(P, 2).astype(np.float32),
    ], axis=1)  # [P, 4C+2] -> appended to params_bf
    fb = np.concatenate([b_blocks(b1), b_blocks(b2)], axis=1)  # [P, 4]
    return wb, fb, float(np.asarray(b3).reshape(-1)[0])


def _build_bass(dims, b3val):
    NT = dims["NT"]
    A_PAD = dims["A_PAD"]
    G = dims["G"]
    E_PAD = dims["E_PAD"]
    NCHUNK = dims["NCHUNK"]
    CPT = dims["CPT"]  # chunks per atom tile
    D = dims["D"]
    A_PAD_CONST = A_PAD
    GC = GROUP_CHUNKS * C  # elementwise group width (1024)
    XC = DMA_GROUPS * GC  # x DMA tile width (4096)

    nc = bacc.Bacc("TRN2", target_bir_lowering=False, debug=False,
                   num_devices=N_CORES)
    xg_d = nc.dram_tensor("xg", [D * P, XC], BF16, kind="ExternalInput")
    rbf_shape = [2 * KF, G * 2 * P] if PACK_FILTER else [KF, E_PAD]
    rbfT_d = nc.dram_tensor("rbfT", rbf_shape, BF16, kind="ExternalInput")
    PBW = P + C + 2 * (2 * C) + 2  # iota | wrbfT | w1T | w2T | w3T
    PFW = NCHUNK + 4  # liT | b1 | b2
    pbf_d = nc.dram_tensor("params_bf", [P, PBW], BF16, kind="ExternalInput")
    pf_d = nc.dram_tensor("params_f32", [P, PFW], F32, kind="ExternalInput")
    y_d = nc.dram_tensor("y", [1, A_PAD], F32, kind="ExternalOutput")

    with tile.TileContext(nc) as tc:
        with (
            tc.tile_pool(name="const", bufs=1) as constp,
            tc.tile_pool(name="pers", bufs=1) as pers,
            tc.tile_pool(name="xt", bufs=4) as xtp,
            tc.tile_pool(name="fsb", bufs=6) as fsbp,
            tc.tile_pool(name="msg", bufs=6) as msgp,
            tc.tile_pool(name="oh", bufs=32) as ohp,
            tc.tile_pool(name="fps", bufs=2, space="PSUM") as fpsp,
            tc.tile_pool(name="sps", bufs=2, space="PSUM") as spsp,
            tc.tile_pool(name="tps", bufs=1, space="PSUM") as tpsp,
            tc.tile_pool(name="mps", bufs=1, space="PSUM") as mpsp,
        ):
            # --- constants: one bundled DMA each for bf16/f32 params ---
            pbf_sb = constp.tile([P, PBW], BF16)
            nc.sync.dma_start(pbf_sb[:], pbf_d[:])
            pf_sb = constp.tile([P, PFW], F32)
            nc.sync.dma_start(pf_sb[:], pf_d[:])
            iota_sb = pbf_sb[:, 0:P]
            wrbfT_sb = pbf_sb[:, P:P + C]
            w1T_sb = pbf_sb[:, P + C:P + C + 2 * C].rearrange(
                "p (k c) -> p k c", k=2)
            w2T_sb = pbf_sb[:, P + 3 * C:P + 5 * C].rearrange(
                "p (k c) -> p k c", k=2)
            w3T_sb = pbf_sb[:, P + 5 * C:P + 5 * C + 2].rearrange(
                "p (k c) -> p k c", k=2)
            liT_sb = pf_sb[:, 0:NCHUNK]
            b1_sb = pf_sb[:, NCHUNK:NCHUNK + 2]
            b2_sb = pf_sb[:, NCHUNK + 2:NCHUNK + 4]
            sb_rbf_shape = [49, rbf_shape[1]] if PACK_FILTER else rbf_shape
            rbfT_sb = constp.tile(sb_rbf_shape, BF16)
            head = min(rbf_shape[1], 2 * (rbf_shape[1] // NT))

            def dma_rbf(c0, c1):
                if PACK_FILTER:
                    nc.sync.dma_start(rbfT_sb[0:KF, c0:c1],
                                      rbfT_d[0:KF, c0:c1])
                    nc.sync.dma_start(rbfT_sb[32:32 + KF, c0:c1],
                                      rbfT_d[KF:2 * KF, c0:c1])
                else:
                    nc.sync.dma_start(rbfT_sb[:, c0:c1], rbfT_d[:, c0:c1])

            dma_rbf(0, head)

            xts = {}
            fpss = {}
            spsums = {}
            mlp_points = {}  # last tile index -> list of n0 chunks ready
            for n0 in range(0, A_PAD_CONST, 512):
                nsz = min(512, A_PAD_CONST - n0)
                t_req = (n0 + nsz - 1) // P
                mlp_points.setdefault(t_req, []).append(n0)

            def emit_dma(d):
                xt = xtp.tile([P, XC], BF16, name="xt", tag="xt")
                nc.sync.dma_start(xt[:], xg_d[d * P:(d + 1) * P, :])
                xts[d] = xt

            def emit_filter(g):
                if PACK_FILTER:
                    fps = fpsp.tile([P, 2, 512], F32, name="fps", tag="fps")
                else:
                    fps = fpsp.tile([P, GC], F32, name="fps", tag="fps")
                for q in range(GROUP_CHUNKS):
                    ch = g * GROUP_CHUNKS + q
                    if PACK_FILTER:
                        rg = 32 * (q % 2)
                        nc.tensor.matmul(
                            fps[:, q % 2, (q // 2) * C:(q // 2 + 1) * C],
                            lhsT=rbfT_sb[rg:rg + KF,
                                         g * 2 * P + (q // 2) * P:
                                         g * 2 * P + (q // 2 + 1) * P],
                            rhs=wrbfT_sb[rg:rg + KF, :],
                            start=True,
                            stop=True,
                            tile_position=(rg, 0),
                        )
                    else:
                        nc.tensor.matmul(
                            fps[:, q * C:(q + 1) * C],
                            lhsT=rbfT_sb[:, ch * P:(ch + 1) * P],
                            rhs=wrbfT_sb[:KF, :],
                            start=True,
                            stop=True,
                        )
                fpss[g] = fps

            def emit_consume(g):
                fps = fpss.pop(g)
                xt = xts[g // DMA_GROUPS]
                g2 = g % DMA_GROUPS
                fsb = fsbp.tile([P, GC], BF16, name="fsb", tag="fsb")
                fps_ap = fps[:] if PACK_FILTER else fps[:]
                if g % 5 == 2:
                    nc.vector.tensor_copy(fsb[:], fps_ap)
                else:
                    nc.scalar.activation(
                        fsb[:], fps_ap, mybir.ActivationFunctionType.Copy,
                    )
                msg = msgp.tile([P, GC], BF16, name="msg", tag="msg")
                nc.vector.tensor_tensor(
                    out=msg[:], in0=fsb[:],
                    in1=xt[:, g2 * GC:(g2 + 1) * GC],
                    op=mybir.AluOpType.mult,
                )
                for q in range(GROUP_CHUNKS):
                    ch = g * GROUP_CHUNKS + q
                    t, ct = divmod(ch, CPT)
                    if ct == 0:
                        spsums[t] = spsp.tile([P, C], F32, name="spsum",
                                              tag="sps")
                    oh = ohp.tile([P, P], BF16, name="oh", tag="oh")
                    oh_eng = nc.vector if ch % 8 == 7 else nc.gpsimd
                    oh_eng.tensor_scalar(
                        oh[:], iota_sb[:], liT_sb[:, ch:ch + 1], None,
                        mybir.AluOpType.is_equal,
                    )
                    pq = (2 * (q % 2) + q // 2) if PACK_FILTER else q
                    nc.tensor.matmul(
                        spsums[t][:],
                        lhsT=oh[:],
                        rhs=msg[:, pq * C:(pq + 1) * C],
                        start=(ct == 0),
                        stop=(ct == CPT - 1),
                    )
                    if ct == CPT - 1:
                        emit_tile_end(t)
                        for n0 in mlp_points.get(t, []):
                            emit_mlp_chunk(n0)

            def emit_tile_end(t):
                nc.any.tensor_copy(h0_all[:, t * C:(t + 1) * C],
                                   spsums.pop(t)[:])
                for k in range(2):
                    tps = tpsp.tile([P, P], BF16, name="tps", tag="tps")
                    nc.tensor.transpose(
                        tps[:],
                        h0_all[:, t * C + k * P: t * C + (k + 1) * P],
                        ident_sb[:],
                    )
                    nc.any.tensor_copy(hT[:, k, t * P:(t + 1) * P], tps[:])

            def emit_mlp_chunk(n0):
                nsz = min(512, A_PAD - n0)

                def layer(src_t, dst, wsb, bsb):
                    mp = mpsp.tile([P, 512], F32, name="mp", tag="mp")
                    for k in range(2):
                        nc.tensor.matmul(
                            mp[:, :nsz],
                            lhsT=wsb[:, k, :] if wsb is w3T_sb
                            else wsb[:, k, 0:P],
                            rhs=src_t[:, k, n0:n0 + nsz],
                            start=(k == 0),
                            stop=(k == 1),
                        )
                    return mp

                for m in range(2):
                    mp = mpsp.tile([P, 512], F32, name="mp", tag="mp")
                    for k in range(2):
                        nc.tensor.matmul(
                            mp[:, :nsz],
                            lhsT=w1T_sb[:, k, m * P:(m + 1) * P],
                            rhs=hT[:, k, n0:n0 + nsz],
                            start=(k == 0), stop=(k == 1),
                        )
                    nc.scalar.activation(
                        h1T[:, m, n0:n0 + nsz], mp[:, :nsz],
                        mybir.ActivationFunctionType.Silu,
                        bias=b1_sb[:, m:m + 1],
                    )
                for m in range(2):
                    mp = mpsp.tile([P, 512], F32, name="mp", tag="mp")
                    for k in range(2):
                        nc.tensor.matmul(
                            mp[:, :nsz],
                            lhsT=w2T_sb[:, k, m * P:(m + 1) * P],
                            rhs=h1T[:, k, n0:n0 + nsz],
                            start=(k == 0), stop=(k == 1),
                        )
                    nc.scalar.activation(
                        h2T[:, m, n0:n0 + nsz], mp[:, :nsz],
                        mybir.ActivationFunctionType.Silu,
                        bias=b2_sb[:, m:m + 1],
                    )
                mp = mpsp.tile([P, 512], F32, name="mp", tag="mp")
                for k in range(2):
                    nc.tensor.matmul(
                        mp[:1, :nsz],
                        lhsT=w3T_sb[:, k, :],
                        rhs=h2T[:, k, n0:n0 + nsz],
                        start=(k == 0), stop=(k == 1),
                    )
                nc.scalar.activation(
                    y_sb[:, n0:n0 + nsz], mp[:1, :nsz],
                    mybir.ActivationFunctionType.Copy, bias=b3val,
                )

            # --- pipelined emission ---
            emit_dma(0)
            emit_filter(0)

            # remaining constants (needed later; after the first x tile)
            if head < rbf_shape[1]:
                dma_rbf(head, rbf_shape[1])
            ident_sb = constp.tile([P, P], BF16)
            make_identity(nc, ident_sb[:])

            h0_all = pers.tile([P, NT * C], BF16)
            hT = pers.tile([P, 2, A_PAD], BF16)
            h1T = pers.tile([P, 2, A_PAD], BF16)
            h2T = pers.tile([P, 2, A_PAD], BF16)
            y_sb = pers.tile([1, A_PAD], F32)

            for g in range(G):
                if (g + 1) % DMA_GROUPS == 0 and g + 1 < G:
                    emit_dma((g + 1) // DMA_GROUPS)
                if g + 1 < G:
                    emit_filter(g + 1)
                emit_consume(g)
            nc.sync.dma_start(y_d[:], y_sb[:])

    nc.compile()
    return nc


def _prepare(x, rbf, num_atoms, edge_index_0, w_rbf, b_rbf, w1, b1, w2, b2, w3, b3):
    x = np.asarray(x, dtype=np.float32)
    rbf = np.asarray(rbf, dtype=np.float32)
    num_atoms = int(num_atoms)
    per_core, shared, dims = _host_prep(x, rbf, num_atoms, edge_index_0,
                                        np.asarray(w_rbf, np.float32),
                                        np.asarray(b_rbf, np.float32))
    wb, fb, b3val = _mlp_weights(
        np.asarray(w1, np.float32), np.asarray(b1, np.float32),
        np.asarray(w2, np.float32), np.asarray(b2, np.float32),
        np.asarray(w3, np.float32), np.asarray(b3, np.float32))
    params_bf = np.concatenate(
        [shared["params_bf"].astype(np.float32), wb], axis=1).astype(NP_BF16)
    nc = _build_bass(dims, b3val)
    in_maps = []
    for pc in per_core:
        params_f32 = np.concatenate([pc["_liT"], fb], axis=1).astype(np.float32)
        in_maps.append({"xg": pc["xg"], "rbfT": pc["rbfT"],
                        "params_bf": params_bf, "params_f32": params_f32})
    return nc, in_maps, dims


def assemble_output(res_y, dims, num_atoms):
    """res_y: list of per-core [1, A_PAD] arrays -> [num_atoms, 1]."""
    NT = dims["NT"]
    ys = np.stack([np.asarray(y)[0] for y in res_y])  # [N_CORES, A_PAD]
    b = dims["bin_of_atom"]
    out = ys[b // NT, (b % NT) * P + dims["pos_of_atom"]]
    return out.reshape(num_atoms, 1).astype(np.float32)


def kernel(**inputs) -> np.ndarray:
    num_atoms = int(inputs["num_atoms"])
    nc, in_maps, dims = _prepare(**inputs)
    res = run_bass_kernel_spmd(nc, in_maps, core_ids=list(range(N_CORES)))
    return assemble_output([r["y"] for r in res.results], dims, num_atoms)


# revision 37
# speedup vs baseline: 579.1652x; 1.0220x over previous
"""Trainium2 Bass kernel for AtomWise GNN message passing.

reference:
    rbf_filter = rbf @ w_rbf.T + b_rbf        # [E, C]
    msg = rbf_filter * x                      # [E, C]
    out = segment_sum(msg, edge_index_0, N)   # [N, C]
    out = silu(out @ w1.T + b1); out = silu(out @ w2.T + b2); out = out @ w3.T + b3

Strategy (8 NeuronCores, no collectives):
  - Host: stable-sort edges by destination atom; shard ATOMS (N/8 per core) so
    each core owns all edges of its atom range.  Within a core, atoms are
    processed in 128-atom tiles; each tile's edge list is padded to a global
    E_TILE so every core runs the identical SPMD program.
  - Device (per core, per 512-edge group):
      PE:  filter = rbf_chunk(K=17, bias row folded) @ w_rbfT -> PSUM
      ACT: evacuate filter PSUM -> SBUF bf16
      DVE: msg = filter * x  (bf16 2x mode)
      DVE: one-hot[e, a] = (iota_row == li[e])  (tensor_scalar is_equal, 4x)
      PE:  atom_psum[a, c] += one-hot.T @ msg   (scatter-add as matmul)
    Then per-atom-tile PSUM -> SBUF, PE transposes to [C, atoms] layout and a
    3-layer MLP (bf16 matmuls, f32 accumulate) runs on-chip; output [1, atoms].
"""

import os as _os

# This kernel executes on the neuron/axon PJRT devices; a JAX_PLATFORMS=cpu
# pin (meant for running jax reference oracles on CPU) would hide them.
if _os.environ.get("JAX_PLATFORMS", "") == "cpu":
    _os.environ.pop("JAX_PLATFORMS")

import numpy as np

import concourse.bacc as bacc
import concourse.mybir as mybir
import concourse.tile as tile
from concourse.bass_utils import run_bass_kernel_spmd
from concourse.masks import make_identity

N_CORES = 8
P = 128
C = 256
RBF = 16
KF = RBF + 1  # rbf channels + bias row
CHUNK = 128  # edges per scatter matmul (contraction dim)
GROUP_CHUNKS = 4
GROUP_E = CHUNK * GROUP_CHUNKS  # 512 edges per elementwise group
DMA_GROUPS = 4  # groups per x DMA (2048 edges, 1 MiB)
DMA_E = GROUP_E * DMA_GROUPS
BF16 = mybir.dt.bfloat16
F32 = mybir.dt.float32
NP_BF16 = mybir.dt.np(BF16)

# tile_position row-packing of the K=17 filter matmuls (2 concurrent row
# groups). Disabled: concurrent row-group matmuls draining into PSUM showed a
# rare NRT_EXEC_UNIT_UNRECOVERABLE device fault when two drains land in the
# same PSUM bank window. The serial path is deterministic and ~10% slower on
# the tensor engine only.
PACK_FILTER = False


def _host_prep(x, rbf, num_atoms, edge_index_0, w_rbf, b_rbf):
    """Sort/shard/pad on host with balanced atom binning.

    Atoms are assigned to N_CORES*NT bins (max P atoms each) by greedy LPT on
    edge count, so every bin has nearly equal edges -> minimal padding. Bin b
    maps to core b // NT, atom-tile b % NT, and an atom's one-hot column is
    its position within the bin. Returns the atom->(bin,pos) maps for output
    reassembly.
    """
    import heapq

    n_local = num_atoms // N_CORES
    assert num_atoms % N_CORES == 0
    NT = (n_local + P - 1) // P  # atom tiles per core
    NBINS = N_CORES * NT

    idx = np.asarray(edge_index_0).astype(np.int64)
    counts = np.bincount(idx, minlength=num_atoms)

    # LPT: biggest atoms first into the least-loaded non-full bin
    bin_of_atom = np.empty(num_atoms, dtype=np.int64)
    pos_of_atom = np.empty(num_atoms, dtype=np.int64)
    bin_fill = np.zeros(NBINS, dtype=np.int64)
    heap = [(0, b) for b in range(NBINS)]
    heapq.heapify(heap)
    atom_order = np.argsort(-counts, kind="stable")
    spill = []
    for a in atom_order:
        while True:
            s, b = heapq.heappop(heap)
            if bin_fill[b] < P:
                break
            spill.append((s, b))
        bin_of_atom[a] = b
        pos_of_atom[a] = bin_fill[b]
        bin_fill[b] += 1
        heapq.heappush(heap, (s + int(counts[a]), b))
        for item in spill:
            heapq.heappush(heap, item)
        spill.clear()

    bin_of_edge = bin_of_atom[idx]
    order_all = np.argsort(bin_of_edge, kind="stable")
    bin_counts = np.bincount(bin_of_edge, minlength=NBINS)
    bin_start = np.concatenate([[0], np.cumsum(bin_counts)])

    E_TILE = int(-(-bin_counts.max() // CHUNK) * CHUNK)
    while (NT * E_TILE) % GROUP_E != 0:
        E_TILE += CHUNK
    E_PAD = NT * E_TILE  # per-core consumed edge slots
    G = E_PAD // GROUP_E
    NCHUNK = E_PAD // CHUNK
    CPT = E_TILE // CHUNK  # chunks per atom tile
    D = -(-G // DMA_GROUPS)  # x DMA count (last may be partly consumed)
    E_XG = D * DMA_E

    per_core = []
    for c in range(N_CORES):
        xs = np.zeros((E_XG, C), dtype=np.float32)
        rbf_pad = np.zeros((E_PAD, KF), dtype=np.float32)
        li = np.full((E_PAD,), -1.0, dtype=np.float32)
        for t in range(NT):
            b = c * NT + t
            order = order_all[bin_start[b]:bin_start[b + 1]]
            n = len(order)
            s = t * E_TILE
            xs[s:s + n] = x[order]
            rbf_pad[s:s + n, :RBF] = rbf[order]
            rbf_pad[s:s + n, RBF] = 1.0
            li[s:s + n] = pos_of_atom[idx[order]].astype(np.float32)

        # x: [D, (4 dma-groups, 4 chunks), 128, C] -> [D*128, 16*C]
        # with PACK_FILTER, chunks within a group are stored in the psum
        # evacuation order [0, 2, 1, 3]
        xs4 = xs.reshape(D, DMA_GROUPS, GROUP_CHUNKS, P, C)
        if PACK_FILTER:
            xs4 = xs4[:, :, [0, 2, 1, 3]]
        xg = (
            xs4.reshape(D, DMA_GROUPS * GROUP_CHUNKS, P, C)
            .transpose(0, 2, 1, 3)
            .reshape(D * P, DMA_GROUPS * GROUP_CHUNKS * C)
            .astype(NP_BF16)
        )
        if PACK_FILTER:
            # rbfT packed for 2-row-group tiling: chunk (g,q) on partitions
            # [KF*(q%2), +KF) of the compact array, cols [g*256+(q//2)*128)
            arr = rbf_pad.reshape(G, GROUP_CHUNKS, P, KF)
            rbfT = np.zeros((2 * KF, G, 2, P), dtype=np.float32)
            for q in range(GROUP_CHUNKS):
                rbfT[KF * (q % 2):KF * (q % 2) + KF, :, q // 2, :] = (
                    arr[:, q].transpose(2, 0, 1)
                )
            rbfT = rbfT.reshape(2 * KF, G * 2 * P).astype(NP_BF16)
        else:
            rbfT = np.ascontiguousarray(rbf_pad.T).astype(NP_BF16)
        liT = np.ascontiguousarray(li.reshape(NCHUNK, P).T)  # f32 [P, NCHUNK]
        per_core.append({"xg": xg, "rbfT": rbfT, "_liT": liT})

    wrbfT = np.concatenate(
        [w_rbf.T.astype(np.float32), b_rbf[None].astype(np.float32)], axis=0
    )  # [KF, C]
    if PACK_FILTER:
        w4 = np.zeros((P, C), dtype=np.float32)
        for rg in (0, 32):
            w4[rg:rg + KF] = wrbfT
        wrbfT = w4
    iota = np.tile(np.arange(P, dtype=np.float32), (P, 1))
    # bf16 bundle: [iota(128) | wrbfT(C)]
    pb = np.concatenate([iota, np.zeros((P, C), np.float32)], axis=1)
    pb[:wrbfT.shape[0], P:P + C] = wrbfT
    shared = {"params_bf": pb.astype(NP_BF16)}
    dims = dict(NT=NT, A_PAD=NT * P, E_TILE=E_TILE, G=G, E_PAD=E_PAD,
                NCHUNK=NCHUNK, CPT=CPT, n_local=n_local, D=D,
                bin_of_atom=bin_of_atom, pos_of_atom=pos_of_atom)
    return per_core, shared, dims


def _mlp_weights(w1, b1, w2, b2, w3, b3):
    def wT_blocks(w):  # w [out, in] -> lhsT blocks [P, in//P, out]
        wt = w.T.astype(np.float32)  # [in, out]
        i_dim, o_dim = wt.shape
        return np.ascontiguousarray(
            wt.reshape(i_dim // P, P, o_dim).transpose(1, 0, 2)
        ).astype(NP_BF16).astype(np.float32)

    def b_blocks(b):  # [out] -> [P, out//P]
        return np.ascontiguousarray(b.astype(np.float32).reshape(-1, P).T)

    wb = np.concatenate([
        wT_blocks(w1).reshape(P, 2 * C).astype(np.float32),
        wT_blocks(w2).reshape(P, 2 * C).astype(np.float32),
        wT_blocks(w3).reshape(P, 2).astype(np.float32),
    ], axis=1)  # [P, 4C+2] -> appended to params_bf
    fb = np.concatenate([b_blocks(b1), b_blocks(b2)], axis=1)  # [P, 4]
    return wb, fb, float(np.asarray(b3).reshape(-1)[0])


def _build_bass(dims, b3val):
    NT = dims["NT"]
    A_PAD = dims["A_PAD"]
    G = dims["G"]
    E_PAD = dims["E_PAD"]
    NCHUNK = dims["NCHUNK"]
    CPT = dims["CPT"]  # chunks per atom tile
    D = dims["D"]
    A_PAD_CONST = A_PAD
    GC = GROUP_CHUNKS * C  # elementwise group width (1024)
    XC = DMA_GROUPS * GC  # x DMA tile width (4096)

    nc = bacc.Bacc("TRN2", target_bir_lowering=False, debug=False,
                   num_devices=N_CORES)
    xg_d = nc.dram_tensor("xg", [D * P, XC], BF16, kind="ExternalInput")
    rbf_shape = [2 * KF, G * 2 * P] if PACK_FILTER else [KF, E_PAD]
    rbfT_d = nc.dram_tensor("rbfT", rbf_shape, BF16, kind="ExternalInput")
    PBW = P + C + 2 * (2 * C) + 2  # iota | wrbfT | w1T | w2T | w3T
    PFW = NCHUNK + 4  # liT | b1 | b2
    pbf_d = nc.dram_tensor("params_bf", [P, PBW], BF16, kind="ExternalInput")
    pf_d = nc.dram_tensor("params_f32", [P, PFW], F32, kind="ExternalInput")
    y_d = nc.dram_tensor("y", [1, A_PAD], F32, kind="ExternalOutput")

    with tile.TileContext(nc) as tc:
        with (
            tc.tile_pool(name="const", bufs=1) as constp,
            tc.tile_pool(name="pers", bufs=1) as pers,
            tc.tile_pool(name="xt", bufs=4) as xtp,
            tc.tile_pool(name="fsb", bufs=6) as fsbp,
            tc.tile_pool(name="msg", bufs=6) as msgp,
            tc.tile_pool(name="oh", bufs=32) as ohp,
            tc.tile_pool(name="fps", bufs=2, space="PSUM") as fpsp,
            tc.tile_pool(name="sps", bufs=2, space="PSUM") as spsp,
            tc.tile_pool(name="tps", bufs=1, space="PSUM") as tpsp,
            tc.tile_pool(name="mps", bufs=1, space="PSUM") as mpsp,
        ):
            # --- constants: one bundled DMA each for bf16/f32 params ---
            pbf_sb = constp.tile([P, PBW], BF16)
            nc.sync.dma_start(pbf_sb[:], pbf_d[:])
            pf_sb = constp.tile([P, PFW], F32)
            nc.sync.dma_start(pf_sb[:], pf_d[:])
            iota_sb = pbf_sb[:, 0:P]
            wrbfT_sb = pbf_sb[:, P:P + C]
            w1T_sb = pbf_sb[:, P + C:P + C + 2 * C].rearrange(
                "p (k c) -> p k c", k=2)
            w2T_sb = pbf_sb[:, P + 3 * C:P + 5 * C].rearrange(
                "p (k c) -> p k c", k=2)
            w3T_sb = pbf_sb[:, P + 5 * C:P + 5 * C + 2].rearrange(
                "p (k c) -> p k c", k=2)
            liT_sb = pf_sb[:, 0:NCHUNK]
            b1_sb = pf_sb[:, NCHUNK:NCHUNK + 2]
            b2_sb = pf_sb[:, NCHUNK + 2:NCHUNK + 4]
            sb_rbf_shape = [49, rbf_shape[1]] if PACK_FILTER else rbf_shape
            rbfT_sb = constp.tile(sb_rbf_shape, BF16)
            head = min(rbf_shape[1], 2 * (rbf_shape[1] // NT))

            def dma_rbf(c0, c1):
                if PACK_FILTER:
                    nc.sync.dma_start(rbfT_sb[0:KF, c0:c1],
                                      rbfT_d[0:KF, c0:c1])
                    nc.sync.dma_start(rbfT_sb[32:32 + KF, c0:c1],
                                      rbfT_d[KF:2 * KF, c0:c1])
                else:
                    nc.sync.dma_start(rbfT_sb[:, c0:c1], rbfT_d[:, c0:c1])

            dma_rbf(0, head)

            xts = {}
            fpss = {}
            spsums = {}
            mlp_points = {}  # last tile index -> list of n0 chunks ready
            for n0 in range(0, A_PAD_CONST, 512):
                nsz = min(512, A_PAD_CONST - n0)
                t_req = (n0 + nsz - 1) // P
                mlp_points.setdefault(t_req, []).append(n0)

            def emit_dma(d):
                xt = xtp.tile([P, XC], BF16, name="xt", tag="xt")
                nc.sync.dma_start(xt[:], xg_d[d * P:(d + 1) * P, :])
                xts[d] = xt

            def emit_filter(g):
                if PACK_FILTER:
                    fps = fpsp.tile([P, 2, 512], F32, name="fps", tag="fps")
                else:
                    fps = fpsp.tile([P, GC], F32, name="fps", tag="fps")
                for q in range(GROUP_CHUNKS):
                    ch = g * GROUP_CHUNKS + q
                    if PACK_FILTER:
                        rg = 32 * (q % 2)
                        nc.tensor.matmul(
                            fps[:, q % 2, (q // 2) * C:(q // 2 + 1) * C],
                            lhsT=rbfT_sb[rg:rg + KF,
                                         g * 2 * P + (q // 2) * P:
                                         g * 2 * P + (q // 2 + 1) * P],
                            rhs=wrbfT_sb[rg:rg + KF, :],
                            start=True,
                            stop=True,
                            tile_position=(rg, 0),
                        )
                    else:
                        nc.tensor.matmul(
                            fps[:, q * C:(q + 1) * C],
                            lhsT=rbfT_sb[:, ch * P:(ch + 1) * P],
                            rhs=wrbfT_sb[:KF, :],
                            start=True,
                            stop=True,
                        )
                fpss[g] = fps

            def emit_consume(g):
                fps = fpss.pop(g)
                xt = xts[g // DMA_GROUPS]
                g2 = g % DMA_GROUPS
                fsb = fsbp.tile([P, GC], BF16, name="fsb", tag="fsb")
                fps_ap = fps[:] if PACK_FILTER else fps[:]
                if g % 5 == 2:
                    nc.vector.tensor_copy(fsb[:], fps_ap)
                else:
                    nc.scalar.activation(
                        fsb[:], fps_ap, mybir.ActivationFunctionType.Copy,
                    )
                msg = msgp.tile([P, GC], BF16, name="msg", tag="msg")
                nc.vector.tensor_tensor(
                    out=msg[:], in0=fsb[:],
                    in1=xt[:, g2 * GC:(g2 + 1) * GC],
                    op=mybir.AluOpType.mult,
                )
                for q in range(GROUP_CHUNKS):
                    ch = g * GROUP_CHUNKS + q
                    t, ct = divmod(ch, CPT)
                    if ct == 0:
                        spsums[t] = spsp.tile([P, C], F32, name="spsum",
                                              tag="sps")
                    oh = ohp.tile([P, P], BF16, name="oh", tag="oh")
                    oh_eng = nc.vector if ch % 8 == 7 else nc.gpsimd
                    oh_eng.tensor_scalar(
                        oh[:], iota_sb[:], liT_sb[:, ch:ch + 1], None,
                        mybir.AluOpType.is_equal,
                    )
                    pq = (2 * (q % 2) + q // 2) if PACK_FILTER else q
                    nc.tensor.matmul(
                        spsums[t][:],
                        lhsT=oh[:],
                        rhs=msg[:, pq * C:(pq + 1) * C],
                        start=(ct == 0),
                        stop=(ct == CPT - 1),
                    )
                    if ct == CPT - 1:
                        emit_tile_end(t)
                        for n0 in mlp_points.get(t, []):
                            emit_mlp_chunk(n0)

            def emit_tile_end(t):
                nc.any.tensor_copy(h0_all[:, t * C:(t + 1) * C],
                                   spsums.pop(t)[:])
                for k in range(2):
                    tps = tpsp.tile([P, P], BF16, name="tps", tag="tps")
                    nc.tensor.transpose(
                        tps[:],
                        h0_all[:, t * C + k * P: t * C + (k + 1) * P],
                        ident_sb[:],
                    )
                    nc.any.tensor_copy(hT[:, k, t * P:(t + 1) * P], tps[:])

            def emit_mlp_chunk(n0):
                nsz = min(512, A_PAD - n0)

                def layer(src_t, dst, wsb, bsb):
                    mp = mpsp.tile([P, 512], F32, name="mp", tag="mp")
                    for k in range(2):
                        nc.tensor.matmul(
                            mp[:, :nsz],
                            lhsT=wsb[:, k, :] if wsb is w3T_sb
                            else wsb[:, k, 0:P],
                            rhs=src_t[:, k, n0:n0 + nsz],
                            start=(k == 0),
                            stop=(k == 1),
                        )
                    return mp

                for m in range(2):
                    mp = mpsp.tile([P, 512], F32, name="mp", tag="mp")
                    for k in range(2):
                        nc.tensor.matmul(
                            mp[:, :nsz],
                            lhsT=w1T_sb[:, k, m * P:(m + 1) * P],
                            rhs=hT[:, k, n0:n0 + nsz],
                            start=(k == 0), stop=(k == 1),
                        )
                    nc.scalar.activation(
                        h1T[:, m, n0:n0 + nsz], mp[:, :nsz],
                        mybir.ActivationFunctionType.Silu,
                        bias=b1_sb[:, m:m + 1],
                    )
                for m in range(2):
                    mp = mpsp.tile([P, 512], F32, name="mp", tag="mp")
                    for k in range(2):
                        nc.tensor.matmul(
                            mp[:, :nsz],
                            lhsT=w2T_sb[:, k, m * P:(m + 1) * P],
                            rhs=h1T[:, k, n0:n0 + nsz],
                            start=(k == 0), stop=(k == 1),
                        )
                    nc.scalar.activation(
                        h2T[:, m, n0:n0 + nsz], mp[:, :nsz],
                        mybir.ActivationFunctionType.Silu,
                        bias=b2_sb[:, m:m + 1],
                    )
                mp = mpsp.tile([P, 512], F32, name="mp", tag="mp")
                for k in range(2):
                    nc.tensor.matmul(
                        mp[:1, :nsz],
                        lhsT=w3T_sb[:, k, :],
                        rhs=h2T[:, k, n0:n0 + nsz],
                        start=(k == 0), stop=(k == 1),
                    )
                nc.scalar.activation(
                    y_sb[:, n0:n0 + nsz], mp[:1, :nsz],
                    mybir.ActivationFunctionType.Copy, bias=b3val,
                )

            # --- pipelined emission (filter runs two groups ahead) ---
            emit_dma(0)
            emit_filter(0)
            emit_filter(1)

            # remaining constants (needed later; after the first x tile)
            if head < rbf_shape[1]:
                dma_rbf(head, rbf_shape[1])
            ident_sb = constp.tile([P, P], BF16)
            make_identity(nc, ident_sb[:])

            h0_all = pers.tile([P, NT * C], BF16)
            hT = pers.tile([P, 2, A_PAD], BF16)
            h1T = pers.tile([P, 2, A_PAD], BF16)
            h2T = pers.tile([P, 2, A_PAD], BF16)
            y_sb = pers.tile([1, A_PAD], F32)

            for g in range(G):
                if (g + 1) % DMA_GROUPS == 0 and g + 1 < G:
                    emit_dma((g + 1) // DMA_GROUPS)
                if g + 2 < G:
                    emit_filter(g + 2)
                emit_consume(g)
            nc.sync.dma_start(y_d[:], y_sb[:])

    nc.compile()
    return nc


def _prepare(x, rbf, num_atoms, edge_index_0, w_rbf, b_rbf, w1, b1, w2, b2, w3, b3):
    x = np.asarray(x, dtype=np.float32)
    rbf = np.asarray(rbf, dtype=np.float32)
    num_atoms = int(num_atoms)
    per_core, shared, dims = _host_prep(x, rbf, num_atoms, edge_index_0,
                                        np.asarray(w_rbf, np.float32),
                                        np.asarray(b_rbf, np.float32))
    wb, fb, b3val = _mlp_weights(
        np.asarray(w1, np.float32), np.asarray(b1, np.float32),
        np.asarray(w2, np.float32), np.asarray(b2, np.float32),
        np.asarray(w3, np.float32), np.asarray(b3, np.float32))
    params_bf = np.concatenate(
        [shared["params_bf"].astype(np.float32), wb], axis=1).astype(NP_BF16)
    nc = _build_bass(dims, b3val)
    in_maps = []
    for pc in per_core:
        params_f32 = np.concatenate([pc["_liT"], fb], axis=1).astype(np.float32)
        in_maps.append({"xg": pc["xg"], "rbfT": pc["rbfT"],
                        "params_bf": params_bf, "params_f32": params_f32})
    return nc, in_maps, dims


def assemble_output(res_y, dims, num_atoms):
    """res_y: list of per-core [1, A_PAD] arrays -> [num_atoms, 1]."""
    NT = dims["NT"]
    ys = np.stack([np.asarray(y)[0] for y in res_y])  # [N_CORES, A_PAD]
    b = dims["bin_of_atom"]
    out = ys[b // NT, (b % NT) * P + dims["pos_of_atom"]]
    return out.reshape(num_atoms, 1).astype(np.float32)


def kernel(**inputs) -> np.ndarray:
    num_atoms = int(inputs["num_atoms"])
    nc, in_maps, dims = _prepare(**inputs)
    res = run_bass_kernel_spmd(nc, in_maps, core_ids=list(range(N_CORES)))
    return assemble_output([r["y"] for r in res.results], dims, num_atoms)


# revision 41
# speedup vs baseline: 584.4473x; 1.0091x over previous
"""Trainium2 Bass kernel for AtomWise GNN message passing.

reference:
    rbf_filter = rbf @ w_rbf.T + b_rbf        # [E, C]
    msg = rbf_filter * x                      # [E, C]
    out = segment_sum(msg, edge_index_0, N)   # [N, C]
    out = silu(out @ w1.T + b1); out = silu(out @ w2.T + b2); out = out @ w3.T + b3

Strategy (8 NeuronCores, no collectives):
  - Host: stable-sort edges by destination atom; shard ATOMS (N/8 per core) so
    each core owns all edges of its atom range.  Within a core, atoms are
    processed in 128-atom tiles; each tile's edge list is padded to a global
    E_TILE so every core runs the identical SPMD program.
  - Device (per core, per 512-edge group):
      PE:  filter = rbf_chunk(K=17, bias row folded) @ w_rbfT -> PSUM
      ACT: evacuate filter PSUM -> SBUF bf16
      DVE: msg = filter * x  (bf16 2x mode)
      DVE: one-hot[e, a] = (iota_row == li[e])  (tensor_scalar is_equal, 4x)
      PE:  atom_psum[a, c] += one-hot.T @ msg   (scatter-add as matmul)
    Then per-atom-tile PSUM -> SBUF, PE transposes to [C, atoms] layout and a
    3-layer MLP (bf16 matmuls, f32 accumulate) runs on-chip; output [1, atoms].
"""

import os as _os

# This kernel executes on the neuron/axon PJRT devices; a JAX_PLATFORMS=cpu
# pin (meant for running jax reference oracles on CPU) would hide them.
if _os.environ.get("JAX_PLATFORMS", "") == "cpu":
    _os.environ.pop("JAX_PLATFORMS")

import numpy as np

import concourse.bacc as bacc
import concourse.mybir as mybir
import concourse.tile as tile
from concourse.bass_utils import run_bass_kernel_spmd
from concourse.masks import make_identity

N_CORES = 8
P = 128
C = 256
RBF = 16
KF = RBF + 1  # rbf channels + bias row
CHUNK = 128  # edges per scatter matmul (contraction dim)
GROUP_CHUNKS = 6
GROUP_E = CHUNK * GROUP_CHUNKS  # 768 edges per elementwise group
DMA_GROUPS = 2  # groups per x DMA (1536 edges, 0.75 MiB)
DMA_E = GROUP_E * DMA_GROUPS
BF16 = mybir.dt.bfloat16
F32 = mybir.dt.float32
NP_BF16 = mybir.dt.np(BF16)

# tile_position row-packing of the K=17 filter matmuls (2 concurrent row
# groups). Disabled: concurrent row-group matmuls draining into PSUM showed a
# rare NRT_EXEC_UNIT_UNRECOVERABLE device fault when two drains land in the
# same PSUM bank window. The serial path is deterministic and ~10% slower on
# the tensor engine only.
PACK_FILTER = False


def _host_prep(x, rbf, num_atoms, edge_index_0, w_rbf, b_rbf):
    """Sort/shard/pad on host with balanced atom binning.

    Atoms are assigned to N_CORES*NT bins (max P atoms each) by greedy LPT on
    edge count, so every bin has nearly equal edges -> minimal padding. Bin b
    maps to core b // NT, atom-tile b % NT, and an atom's one-hot column is
    its position within the bin. Returns the atom->(bin,pos) maps for output
    reassembly.
    """
    import heapq

    n_local = num_atoms // N_CORES
    assert num_atoms % N_CORES == 0
    NT = (n_local + P - 1) // P  # atom tiles per core
    NBINS = N_CORES * NT

    idx = np.asarray(edge_index_0).astype(np.int64)
    counts = np.bincount(idx, minlength=num_atoms)

    # LPT: biggest atoms first into the least-loaded non-full bin
    bin_of_atom = np.empty(num_atoms, dtype=np.int64)
    pos_of_atom = np.empty(num_atoms, dtype=np.int64)
    bin_fill = np.zeros(NBINS, dtype=np.int64)
    heap = [(0, b) for b in range(NBINS)]
    heapq.heapify(heap)
    atom_order = np.argsort(-counts, kind="stable")
    spill = []
    for a in atom_order:
        while True:
            s, b = heapq.heappop(heap)
            if bin_fill[b] < P:
                break
            spill.append((s, b))
        bin_of_atom[a] = b
        pos_of_atom[a] = bin_fill[b]
        bin_fill[b] += 1
        heapq.heappush(heap, (s + int(counts[a]), b))
        for item in spill:
            heapq.heappush(heap, item)
        spill.clear()

    bin_of_edge = bin_of_atom[idx]
    order_all = np.argsort(bin_of_edge, kind="stable")
    bin_counts = np.bincount(bin_of_edge, minlength=NBINS)
    bin_start = np.concatenate([[0], np.cumsum(bin_counts)])

    E_TILE = int(-(-bin_counts.max() // CHUNK) * CHUNK)
    while (NT * E_TILE) % GROUP_E != 0:
        E_TILE += CHUNK
    E_PAD = NT * E_TILE  # per-core consumed edge slots
    G = E_PAD // GROUP_E
    NCHUNK = E_PAD // CHUNK
    CPT = E_TILE // CHUNK  # chunks per atom tile
    D = -(-G // DMA_GROUPS)  # x DMA count (last may be partly consumed)
    E_XG = D * DMA_E

    per_core = []
    for c in range(N_CORES):
        xs = np.zeros((E_XG, C), dtype=np.float32)
        rbf_pad = np.zeros((E_PAD, KF), dtype=np.float32)
        li = np.full((E_PAD,), -1.0, dtype=np.float32)
        for t in range(NT):
            b = c * NT + t
            order = order_all[bin_start[b]:bin_start[b + 1]]
            n = len(order)
            s = t * E_TILE
            xs[s:s + n] = x[order]
            rbf_pad[s:s + n, :RBF] = rbf[order]
            rbf_pad[s:s + n, RBF] = 1.0
            li[s:s + n] = pos_of_atom[idx[order]].astype(np.float32)

        # x: [D, (4 dma-groups, 4 chunks), 128, C] -> [D*128, 16*C]
        # with PACK_FILTER, chunks within a group are stored in the psum
        # evacuation order [0, 2, 1, 3]
        xs4 = xs.reshape(D, DMA_GROUPS, GROUP_CHUNKS, P, C)
        if PACK_FILTER:
            xs4 = xs4[:, :, [0, 2, 1, 3]]
        xg = (
            xs4.reshape(D, DMA_GROUPS * GROUP_CHUNKS, P, C)
            .transpose(0, 2, 1, 3)
            .reshape(D * P, DMA_GROUPS * GROUP_CHUNKS * C)
            .astype(NP_BF16)
        )
        if PACK_FILTER:
            # rbfT packed for 2-row-group tiling: chunk (g,q) on partitions
            # [KF*(q%2), +KF) of the compact array, cols [g*256+(q//2)*128)
            arr = rbf_pad.reshape(G, GROUP_CHUNKS, P, KF)
            rbfT = np.zeros((2 * KF, G, 2, P), dtype=np.float32)
            for q in range(GROUP_CHUNKS):
                rbfT[KF * (q % 2):KF * (q % 2) + KF, :, q // 2, :] = (
                    arr[:, q].transpose(2, 0, 1)
                )
            rbfT = rbfT.reshape(2 * KF, G * 2 * P).astype(NP_BF16)
        else:
            rbfT = np.ascontiguousarray(rbf_pad.T).astype(NP_BF16)
        liT = np.ascontiguousarray(li.reshape(NCHUNK, P).T)  # f32 [P, NCHUNK]
        per_core.append({"xg": xg, "rbfT": rbfT, "_liT": liT})

    wrbfT = np.concatenate(
        [w_rbf.T.astype(np.float32), b_rbf[None].astype(np.float32)], axis=0
    )  # [KF, C]
    if PACK_FILTER:
        w4 = np.zeros((P, C), dtype=np.float32)
        for rg in (0, 32):
            w4[rg:rg + KF] = wrbfT
        wrbfT = w4
    iota = np.tile(np.arange(P, dtype=np.float32), (P, 1))
    # bf16 bundle: [iota(128) | wrbfT(C)]
    pb = np.concatenate([iota, np.zeros((P, C), np.float32)], axis=1)
    pb[:wrbfT.shape[0], P:P + C] = wrbfT
    shared = {"params_bf": pb.astype(NP_BF16)}
    dims = dict(NT=NT, A_PAD=NT * P, E_TILE=E_TILE, G=G, E_PAD=E_PAD,
                NCHUNK=NCHUNK, CPT=CPT, n_local=n_local, D=D,
                bin_of_atom=bin_of_atom, pos_of_atom=pos_of_atom)
    return per_core, shared, dims


def _mlp_weights(w1, b1, w2, b2, w3, b3):
    def wT_blocks(w):  # w [out, in] -> lhsT blocks [P, in//P, out]
        wt = w.T.astype(np.float32)  # [in, out]
        i_dim, o_dim = wt.shape
        return np.ascontiguousarray(
            wt.reshape(i_dim // P, P, o_dim).transpose(1, 0, 2)
        ).astype(NP_BF16).astype(np.float32)

    def b_blocks(b):  # [out] -> [P, out//P]
        return np.ascontiguousarray(b.astype(np.float32).reshape(-1, P).T)

    wb = np.concatenate([
        wT_blocks(w1).reshape(P, 2 * C).astype(np.float32),
        wT_blocks(w2).reshape(P, 2 * C).astype(np.float32),
        wT_blocks(w3).reshape(P, 2).astype(np.float32),
    ], axis=1)  # [P, 4C+2] -> appended to params_bf
    fb = np.concatenate([b_blocks(b1), b_blocks(b2)], axis=1)  # [P, 4]
    return wb, fb, float(np.asarray(b3).reshape(-1)[0])


def _build_bass(dims, b3val):
    NT = dims["NT"]
    A_PAD = dims["A_PAD"]
    G = dims["G"]
    E_PAD = dims["E_PAD"]
    NCHUNK = dims["NCHUNK"]
    CPT = dims["CPT"]  # chunks per atom tile
    D = dims["D"]
    A_PAD_CONST = A_PAD
    GC = GROUP_CHUNKS * C  # elementwise group width (1024)
    XC = DMA_GROUPS * GC  # x DMA tile width (4096)

    nc = bacc.Bacc("TRN2", target_bir_lowering=False, debug=False,
                   num_devices=N_CORES)
    xg_d = nc.dram_tensor("xg", [D * P, XC], BF16, kind="ExternalInput")
    rbf_shape = [2 * KF, G * 2 * P] if PACK_FILTER else [KF, E_PAD]
    rbfT_d = nc.dram_tensor("rbfT", rbf_shape, BF16, kind="ExternalInput")
    PBW = P + C + 2 * (2 * C) + 2  # iota | wrbfT | w1T | w2T | w3T
    PFW = NCHUNK + 4  # liT | b1 | b2
    pbf_d = nc.dram_tensor("params_bf", [P, PBW], BF16, kind="ExternalInput")
    pf_d = nc.dram_tensor("params_f32", [P, PFW], F32, kind="ExternalInput")
    y_d = nc.dram_tensor("y", [1, A_PAD], F32, kind="ExternalOutput")

    with tile.TileContext(nc) as tc:
        with (
            tc.tile_pool(name="const", bufs=1) as constp,
            tc.tile_pool(name="pers", bufs=1) as pers,
            tc.tile_pool(name="xt", bufs=4) as xtp,
            tc.tile_pool(name="fsb", bufs=6) as fsbp,
            tc.tile_pool(name="msg", bufs=6) as msgp,
            tc.tile_pool(name="oh", bufs=32) as ohp,
            tc.tile_pool(name="fps", bufs=2, space="PSUM") as fpsp,
            tc.tile_pool(name="aux", bufs=2, space="PSUM") as auxp,
        ):
            # --- constants: one bundled DMA each for bf16/f32 params ---
            pbf_sb = constp.tile([P, PBW], BF16)
            nc.sync.dma_start(pbf_sb[:], pbf_d[:])
            pf_sb = constp.tile([P, PFW], F32)
            nc.sync.dma_start(pf_sb[:], pf_d[:])
            iota_sb = pbf_sb[:, 0:P]
            wrbfT_sb = pbf_sb[:, P:P + C]
            w1T_sb = pbf_sb[:, P + C:P + C + 2 * C].rearrange(
                "p (k c) -> p k c", k=2)
            w2T_sb = pbf_sb[:, P + 3 * C:P + 5 * C].rearrange(
                "p (k c) -> p k c", k=2)
            w3T_sb = pbf_sb[:, P + 5 * C:P + 5 * C + 2].rearrange(
                "p (k c) -> p k c", k=2)
            liT_sb = pf_sb[:, 0:NCHUNK]
            b1_sb = pf_sb[:, NCHUNK:NCHUNK + 2]
            b2_sb = pf_sb[:, NCHUNK + 2:NCHUNK + 4]
            sb_rbf_shape = [49, rbf_shape[1]] if PACK_FILTER else rbf_shape
            rbfT_sb = constp.tile(sb_rbf_shape, BF16)
            head = min(rbf_shape[1], 2 * (rbf_shape[1] // NT))

            def dma_rbf(c0, c1):
                if PACK_FILTER:
                    nc.sync.dma_start(rbfT_sb[0:KF, c0:c1],
                                      rbfT_d[0:KF, c0:c1])
                    nc.sync.dma_start(rbfT_sb[32:32 + KF, c0:c1],
                                      rbfT_d[KF:2 * KF, c0:c1])
                else:
                    nc.sync.dma_start(rbfT_sb[:, c0:c1], rbfT_d[:, c0:c1])

            dma_rbf(0, head)

            xts = {}
            fpss = {}
            spsums = {}
            mlp_points = {}  # last tile index -> list of n0 chunks ready
            for n0 in range(0, A_PAD_CONST, 512):
                nsz = min(512, A_PAD_CONST - n0)
                t_req = (n0 + nsz - 1) // P
                mlp_points.setdefault(t_req, []).append(n0)

            def emit_dma(d):
                xt = xtp.tile([P, XC], BF16, name="xt", tag="xt")
                nc.sync.dma_start(xt[:], xg_d[d * P:(d + 1) * P, :])
                xts[d] = xt

            def emit_filter(g):
                if PACK_FILTER:
                    fps = fpsp.tile([P, 2, 512], F32, name="fps", tag="fps")
                else:
                    fps = fpsp.tile([P, GC], F32, name="fps", tag="fps")
                for q in range(GROUP_CHUNKS):
                    ch = g * GROUP_CHUNKS + q
                    if PACK_FILTER:
                        rg = 32 * (q % 2)
                        nc.tensor.matmul(
                            fps[:, q % 2, (q // 2) * C:(q // 2 + 1) * C],
                            lhsT=rbfT_sb[rg:rg + KF,
                                         g * 2 * P + (q // 2) * P:
                                         g * 2 * P + (q // 2 + 1) * P],
                            rhs=wrbfT_sb[rg:rg + KF, :],
                            start=True,
                            stop=True,
                            tile_position=(rg, 0),
                        )
                    else:
                        nc.tensor.matmul(
                            fps[:, q * C:(q + 1) * C],
                            lhsT=rbfT_sb[:, ch * P:(ch + 1) * P],
                            rhs=wrbfT_sb[:KF, :],
                            start=True,
                            stop=True,
                        )
                fpss[g] = fps

            def emit_consume(g):
                fps = fpss.pop(g)
                xt = xts[g // DMA_GROUPS]
                g2 = g % DMA_GROUPS
                fsb = fsbp.tile([P, GC], BF16, name="fsb", tag="fsb")
                fps_ap = fps[:] if PACK_FILTER else fps[:]
                if g % 5 == 2:
                    nc.vector.tensor_copy(fsb[:], fps_ap)
                else:
                    nc.scalar.activation(
                        fsb[:], fps_ap, mybir.ActivationFunctionType.Copy,
                    )
                msg = msgp.tile([P, GC], BF16, name="msg", tag="msg")
                nc.vector.tensor_tensor(
                    out=msg[:], in0=fsb[:],
                    in1=xt[:, g2 * GC:(g2 + 1) * GC],
                    op=mybir.AluOpType.mult,
                )
                for q in range(GROUP_CHUNKS):
                    ch = g * GROUP_CHUNKS + q
                    t, ct = divmod(ch, CPT)
                    if ct == 0:
                        spsums[t] = auxp.tile([P, 512], F32, name="spsum",
                                              tag="aux")[:, :C]
                    oh = ohp.tile([P, P], BF16, name="oh", tag="oh")
                    oh_eng = nc.vector if ch % 8 == 7 else nc.gpsimd
                    oh_eng.tensor_scalar(
                        oh[:], iota_sb[:], liT_sb[:, ch:ch + 1], None,
                        mybir.AluOpType.is_equal,
                    )
                    pq = (2 * (q % 2) + q // 2) if PACK_FILTER else q
                    nc.tensor.matmul(
                        spsums[t][:],
                        lhsT=oh[:],
                        rhs=msg[:, pq * C:(pq + 1) * C],
                        start=(ct == 0),
                        stop=(ct == CPT - 1),
                    )
                    if ct == CPT - 1:
                        if t + 1 < NT and t + 1 not in spsums:
                            spsums[t + 1] = auxp.tile(
                                [P, 512], F32, name="spsum", tag="aux")[:, :C]
                        emit_tile_end(t)
                        for n0 in mlp_points.get(t, []):
                            emit_mlp_chunk(n0)

            def emit_tile_end(t):
                nc.any.tensor_copy(h0_all[:, t * C:(t + 1) * C],
                                   spsums.pop(t)[:])
                for k in range(2):
                    tps = auxp.tile([P, P], BF16, name="tps", tag="aux")
                    nc.tensor.transpose(
                        tps[:],
                        h0_all[:, t * C + k * P: t * C + (k + 1) * P],
                        ident_sb[:],
                    )
                    nc.any.tensor_copy(hT[:, k, t * P:(t + 1) * P], tps[:])

            def emit_mlp_chunk(n0):
                nsz = min(512, A_PAD - n0)

                def layer(src_t, dst, wsb, bsb):
                    mp = auxp.tile([P, 512], F32, name="mp", tag="aux")
                    for k in range(2):
                        nc.tensor.matmul(
                            mp[:, :nsz],
                            lhsT=wsb[:, k, :] if wsb is w3T_sb
                            else wsb[:, k, 0:P],
                            rhs=src_t[:, k, n0:n0 + nsz],
                            start=(k == 0),
                            stop=(k == 1),
                        )
                    return mp

                for m in range(2):
                    mp = auxp.tile([P, 512], F32, name="mp", tag="aux")
                    for k in range(2):
                        nc.tensor.matmul(
                            mp[:, :nsz],
                            lhsT=w1T_sb[:, k, m * P:(m + 1) * P],
                            rhs=hT[:, k, n0:n0 + nsz],
                            start=(k == 0), stop=(k == 1),
                        )
                    nc.scalar.activation(
                        h1T[:, m, n0:n0 + nsz], mp[:, :nsz],
                        mybir.ActivationFunctionType.Silu,
                        bias=b1_sb[:, m:m + 1],
                    )
                for m in range(2):
                    mp = auxp.tile([P, 512], F32, name="mp", tag="aux")
                    for k in range(2):
                        nc.tensor.matmul(
                            mp[:, :nsz],
                            lhsT=w2T_sb[:, k, m * P:(m + 1) * P],
                            rhs=h1T[:, k, n0:n0 + nsz],
                            start=(k == 0), stop=(k == 1),
                        )
                    nc.scalar.activation(
                        h2T[:, m, n0:n0 + nsz], mp[:, :nsz],
                        mybir.ActivationFunctionType.Silu,
                        bias=b2_sb[:, m:m + 1],
                    )
                mp = auxp.tile([P, 512], F32, name="mp", tag="aux")
                for k in range(2):
                    nc.tensor.matmul(
                        mp[:1, :nsz],
                        lhsT=w3T_sb[:, k, :],
                        rhs=h2T[:, k, n0:n0 + nsz],
                        start=(k == 0), stop=(k == 1),
                    )
                nc.scalar.activation(
                    y_sb[:, n0:n0 + nsz], mp[:1, :nsz],
                    mybir.ActivationFunctionType.Copy, bias=b3val,
                )

            # --- pipelined emission (filter runs two groups ahead) ---
            emit_dma(0)
            emit_filter(0)
            emit_filter(1)

            # remaining constants (needed later; after the first x tile)
            if head < rbf_shape[1]:
                dma_rbf(head, rbf_shape[1])
            ident_sb = constp.tile([P, P], BF16)
            make_identity(nc, ident_sb[:])

            h0_all = pers.tile([P, NT * C], BF16)
            hT = pers.tile([P, 2, A_PAD], BF16)
            h1T = pers.tile([P, 2, A_PAD], BF16)
            h2T = pers.tile([P, 2, A_PAD], BF16)
            y_sb = pers.tile([1, A_PAD], F32)

            for g in range(G):
                if (g + 1) % DMA_GROUPS == 0 and g + 1 < G:
                    emit_dma((g + 1) // DMA_GROUPS)
                if g + 2 < G:
                    emit_filter(g + 2)
                emit_consume(g)
            nc.sync.dma_start(y_d[:], y_sb[:])

    nc.compile()
    return nc


def _prepare(x, rbf, num_atoms, edge_index_0, w_rbf, b_rbf, w1, b1, w2, b2, w3, b3):
    x = np.asarray(x, dtype=np.float32)
    rbf = np.asarray(rbf, dtype=np.float32)
    num_atoms = int(num_atoms)
    per_core, shared, dims = _host_prep(x, rbf, num_atoms, edge_index_0,
                                        np.asarray(w_rbf, np.float32),
                                        np.asarray(b_rbf, np.float32))
    wb, fb, b3val = _mlp_weights(
        np.asarray(w1, np.float32), np.asarray(b1, np.float32),
        np.asarray(w2, np.float32), np.asarray(b2, np.float32),
        np.asarray(w3, np.float32), np.asarray(b3, np.float32))
    params_bf = np.concatenate(
        [shared["params_bf"].astype(np.float32), wb], axis=1).astype(NP_BF16)
    nc = _build_bass(dims, b3val)
    in_maps = []
    for pc in per_core:
        params_f32 = np.concatenate([pc["_liT"], fb], axis=1).astype(np.float32)
        in_maps.append({"xg": pc["xg"], "rbfT": pc["rbfT"],
                        "params_bf": params_bf, "params_f32": params_f32})
    return nc, in_maps, dims


def assemble_output(res_y, dims, num_atoms):
    """res_y: list of per-core [1, A_PAD] arrays -> [num_atoms, 1]."""
    NT = dims["NT"]
    ys = np.stack([np.asarray(y)[0] for y in res_y])  # [N_CORES, A_PAD]
    b = dims["bin_of_atom"]
    out = ys[b // NT, (b % NT) * P + dims["pos_of_atom"]]
    return out.reshape(num_atoms, 1).astype(np.float32)


def kernel(**inputs) -> np.ndarray:
    num_atoms = int(inputs["num_atoms"])
    nc, in_maps, dims = _prepare(**inputs)
    res = run_bass_kernel_spmd(nc, in_maps, core_ids=list(range(N_CORES)))
    return assemble_output([r["y"] for r in res.results], dims, num_atoms)


# revision 45
# speedup vs baseline: 589.9122x; 1.0094x over previous
"""Trainium2 Bass kernel for AtomWise GNN message passing.

reference:
    rbf_filter = rbf @ w_rbf.T + b_rbf        # [E, C]
    msg = rbf_filter * x                      # [E, C]
    out = segment_sum(msg, edge_index_0, N)   # [N, C]
    out = silu(out @ w1.T + b1); out = silu(out @ w2.T + b2); out = out @ w3.T + b3

Strategy (8 NeuronCores, no collectives):
  - Host: stable-sort edges by destination atom; shard ATOMS (N/8 per core) so
    each core owns all edges of its atom range.  Within a core, atoms are
    processed in 128-atom tiles; each tile's edge list is padded to a global
    E_TILE so every core runs the identical SPMD program.
  - Device (per core, per 512-edge group):
      PE:  filter = rbf_chunk(K=17, bias row folded) @ w_rbfT -> PSUM
      ACT: evacuate filter PSUM -> SBUF bf16
      DVE: msg = filter * x  (bf16 2x mode)
      DVE: one-hot[e, a] = (iota_row == li[e])  (tensor_scalar is_equal, 4x)
      PE:  atom_psum[a, c] += one-hot.T @ msg   (scatter-add as matmul)
    Then per-atom-tile PSUM -> SBUF, PE transposes to [C, atoms] layout and a
    3-layer MLP (bf16 matmuls, f32 accumulate) runs on-chip; output [1, atoms].
"""

import os as _os

# This kernel executes on the neuron/axon PJRT devices; a JAX_PLATFORMS=cpu
# pin (meant for running jax reference oracles on CPU) would hide them.
if _os.environ.get("JAX_PLATFORMS", "") == "cpu":
    _os.environ.pop("JAX_PLATFORMS")

import numpy as np

import concourse.bacc as bacc
import concourse.mybir as mybir
import concourse.tile as tile
from concourse.bass_utils import run_bass_kernel_spmd
from concourse.masks import make_identity

N_CORES = 8
P = 128
C = 256
RBF = 16
KF = RBF + 1  # rbf channels + bias row
CHUNK = 128  # edges per scatter matmul (contraction dim)
GROUP_CHUNKS = 6
GROUP_E = CHUNK * GROUP_CHUNKS  # 768 edges per elementwise group
DMA_GROUPS = 2  # groups per x DMA (1536 edges, 0.75 MiB)
DMA_E = GROUP_E * DMA_GROUPS
BF16 = mybir.dt.bfloat16
F32 = mybir.dt.float32
NP_BF16 = mybir.dt.np(BF16)

# tile_position row-packing of the K=17 filter matmuls (2 concurrent row
# groups). Disabled: concurrent row-group matmuls draining into PSUM showed a
# rare NRT_EXEC_UNIT_UNRECOVERABLE device fault when two drains land in the
# same PSUM bank window. The serial path is deterministic and ~10% slower on
# the tensor engine only.
PACK_FILTER = False


def _host_prep(x, rbf, num_atoms, edge_index_0, w_rbf, b_rbf):
    """Sort/shard/pad on host with balanced atom binning.

    Atoms are assigned to N_CORES*NT bins (max P atoms each) by greedy LPT on
    edge count, so every bin has nearly equal edges -> minimal padding. Bin b
    maps to core b // NT, atom-tile b % NT, and an atom's one-hot column is
    its position within the bin. Returns the atom->(bin,pos) maps for output
    reassembly.
    """
    import heapq

    n_local = num_atoms // N_CORES
    assert num_atoms % N_CORES == 0
    NT = (n_local + P - 1) // P  # atom tiles per core
    NBINS = N_CORES * NT

    idx = np.asarray(edge_index_0).astype(np.int64)
    counts = np.bincount(idx, minlength=num_atoms)

    # LPT: biggest atoms first into the least-loaded non-full bin
    bin_of_atom = np.empty(num_atoms, dtype=np.int64)
    pos_of_atom = np.empty(num_atoms, dtype=np.int64)
    bin_fill = np.zeros(NBINS, dtype=np.int64)
    heap = [(0, b) for b in range(NBINS)]
    heapq.heapify(heap)
    atom_order = np.argsort(-counts, kind="stable")
    spill = []
    for a in atom_order:
        while True:
            s, b = heapq.heappop(heap)
            if bin_fill[b] < P:
                break
            spill.append((s, b))
        bin_of_atom[a] = b
        pos_of_atom[a] = bin_fill[b]
        bin_fill[b] += 1
        heapq.heappush(heap, (s + int(counts[a]), b))
        for item in spill:
            heapq.heappush(heap, item)
        spill.clear()

    bin_of_edge = bin_of_atom[idx]
    order_all = np.argsort(bin_of_edge, kind="stable")
    bin_counts = np.bincount(bin_of_edge, minlength=NBINS)
    bin_start = np.concatenate([[0], np.cumsum(bin_counts)])

    E_TILE = int(-(-bin_counts.max() // CHUNK) * CHUNK)
    while (NT * E_TILE) % GROUP_E != 0:
        E_TILE += CHUNK
    E_PAD = NT * E_TILE  # per-core consumed edge slots
    G = E_PAD // GROUP_E
    NCHUNK = E_PAD // CHUNK
    CPT = E_TILE // CHUNK  # chunks per atom tile
    D = -(-G // DMA_GROUPS)  # x DMA count (last may be partly consumed)
    E_XG = D * DMA_E

    per_core = []
    for c in range(N_CORES):
        xs = np.zeros((E_XG, C), dtype=np.float32)
        rbf_pad = np.zeros((E_PAD, KF), dtype=np.float32)
        li = np.full((E_PAD,), -1.0, dtype=np.float32)
        for t in range(NT):
            b = c * NT + t
            order = order_all[bin_start[b]:bin_start[b + 1]]
            n = len(order)
            s = t * E_TILE
            xs[s:s + n] = x[order]
            rbf_pad[s:s + n, :RBF] = rbf[order]
            rbf_pad[s:s + n, RBF] = 1.0
            li[s:s + n] = pos_of_atom[idx[order]].astype(np.float32)

        # x: [D, (4 dma-groups, 4 chunks), 128, C] -> [D*128, 16*C]
        # with PACK_FILTER, chunks within a group are stored in the psum
        # evacuation order [0, 2, 1, 3]
        xs4 = xs.reshape(D, DMA_GROUPS, GROUP_CHUNKS, P, C)
        if PACK_FILTER:
            xs4 = xs4[:, :, [0, 2, 1, 3]]
        xg = (
            xs4.reshape(D, DMA_GROUPS * GROUP_CHUNKS, P, C)
            .transpose(0, 2, 1, 3)
            .reshape(D * P, DMA_GROUPS * GROUP_CHUNKS * C)
            .astype(NP_BF16)
        )
        if PACK_FILTER:
            # rbfT packed for 2-row-group tiling: chunk (g,q) on partitions
            # [KF*(q%2), +KF) of the compact array, cols [g*256+(q//2)*128)
            arr = rbf_pad.reshape(G, GROUP_CHUNKS, P, KF)
            rbfT = np.zeros((2 * KF, G, 2, P), dtype=np.float32)
            for q in range(GROUP_CHUNKS):
                rbfT[KF * (q % 2):KF * (q % 2) + KF, :, q // 2, :] = (
                    arr[:, q].transpose(2, 0, 1)
                )
            rbfT = rbfT.reshape(2 * KF, G * 2 * P).astype(NP_BF16)
        else:
            rbfT = np.ascontiguousarray(rbf_pad.T).astype(NP_BF16)
        liT = np.ascontiguousarray(li.reshape(NCHUNK, P).T)  # f32 [P, NCHUNK]
        per_core.append({"xg": xg, "rbfT": rbfT, "_liT": liT})

    wrbfT = np.concatenate(
        [w_rbf.T.astype(np.float32), b_rbf[None].astype(np.float32)], axis=0
    )  # [KF, C]
    if PACK_FILTER:
        w4 = np.zeros((P, C), dtype=np.float32)
        for rg in (0, 32):
            w4[rg:rg + KF] = wrbfT
        wrbfT = w4
    iota = np.tile(np.arange(P, dtype=np.float32), (P, 1))
    # bf16 bundle: [iota(128) | wrbfT(C)]
    pb = np.concatenate([iota, np.zeros((P, C), np.float32)], axis=1)
    pb[:wrbfT.shape[0], P:P + C] = wrbfT
    shared = {"params_bf": pb.astype(NP_BF16)}
    dims = dict(NT=NT, A_PAD=NT * P, E_TILE=E_TILE, G=G, E_PAD=E_PAD,
                NCHUNK=NCHUNK, CPT=CPT, n_local=n_local, D=D,
                bin_of_atom=bin_of_atom, pos_of_atom=pos_of_atom)
    return per_core, shared, dims


def _mlp_weights(w1, b1, w2, b2, w3, b3):
    def wT_blocks(w):  # w [out, in] -> lhsT blocks [P, in//P, out]
        wt = w.T.astype(np.float32)  # [in, out]
        i_dim, o_dim = wt.shape
        return np.ascontiguousarray(
            wt.reshape(i_dim // P, P, o_dim).transpose(1, 0, 2)
        ).astype(NP_BF16).astype(np.float32)

    def b_blocks(b):  # [out] -> [P, out//P]
        return np.ascontiguousarray(b.astype(np.float32).reshape(-1, P).T)

    wb = np.concatenate([
        wT_blocks(w1).reshape(P, 2 * C).astype(np.float32),
        wT_blocks(w2).reshape(P, 2 * C).astype(np.float32),
        wT_blocks(w3).reshape(P, 2).astype(np.float32),
    ], axis=1)  # [P, 4C+2] -> appended to params_bf
    fb = np.concatenate([b_blocks(b1), b_blocks(b2)], axis=1)  # [P, 4]
    return wb, fb, float(np.asarray(b3).reshape(-1)[0])


def _build_bass(dims, b3val):
    NT = dims["NT"]
    A_PAD = dims["A_PAD"]
    G = dims["G"]
    E_PAD = dims["E_PAD"]
    NCHUNK = dims["NCHUNK"]
    CPT = dims["CPT"]  # chunks per atom tile
    D = dims["D"]
    A_PAD_CONST = A_PAD
    GC = GROUP_CHUNKS * C  # elementwise group width (1024)
    XC = DMA_GROUPS * GC  # x DMA tile width (4096)

    nc = bacc.Bacc("TRN2", target_bir_lowering=False, debug=False,
                   num_devices=N_CORES)
    xg_d = nc.dram_tensor("xg", [D * P, XC], BF16, kind="ExternalInput")
    rbf_shape = [2 * KF, G * 2 * P] if PACK_FILTER else [KF, E_PAD]
    rbfT_d = nc.dram_tensor("rbfT", rbf_shape, BF16, kind="ExternalInput")
    PBW = P + C + 2 * (2 * C) + 2  # iota | wrbfT | w1T | w2T | w3T
    PFW = NCHUNK + 4  # liT | b1 | b2
    pbf_d = nc.dram_tensor("params_bf", [P, PBW], BF16, kind="ExternalInput")
    pf_d = nc.dram_tensor("params_f32", [P, PFW], F32, kind="ExternalInput")
    y_d = nc.dram_tensor("y", [1, A_PAD], F32, kind="ExternalOutput")

    with tile.TileContext(nc) as tc:
        with (
            tc.tile_pool(name="const", bufs=1) as constp,
            tc.tile_pool(name="pers", bufs=1) as pers,
            tc.tile_pool(name="xt", bufs=4) as xtp,
            tc.tile_pool(name="fsb", bufs=6) as fsbp,
            tc.tile_pool(name="msg", bufs=6) as msgp,
            tc.tile_pool(name="oh", bufs=32) as ohp,
            tc.tile_pool(name="fps", bufs=2, space="PSUM") as fpsp,
            tc.tile_pool(name="aux", bufs=2, space="PSUM") as auxp,
        ):
            # --- constants: one bundled DMA each for bf16/f32 params ---
            pbf_sb = constp.tile([P, PBW], BF16)
            nc.sync.dma_start(pbf_sb[:], pbf_d[:])
            pf_sb = constp.tile([P, PFW], F32)
            iota_sb = pbf_sb[:, 0:P]
            wrbfT_sb = pbf_sb[:, P:P + C]
            w1T_sb = pbf_sb[:, P + C:P + C + 2 * C].rearrange(
                "p (k c) -> p k c", k=2)
            w2T_sb = pbf_sb[:, P + 3 * C:P + 5 * C].rearrange(
                "p (k c) -> p k c", k=2)
            w3T_sb = pbf_sb[:, P + 5 * C:P + 5 * C + 2].rearrange(
                "p (k c) -> p k c", k=2)
            liT_sb = pf_sb[:, 0:NCHUNK]
            b1_sb = pf_sb[:, NCHUNK:NCHUNK + 2]
            b2_sb = pf_sb[:, NCHUNK + 2:NCHUNK + 4]
            sb_rbf_shape = [49, rbf_shape[1]] if PACK_FILTER else rbf_shape
            rbfT_sb = constp.tile(sb_rbf_shape, BF16)
            head = min(rbf_shape[1], rbf_shape[1] // NT)

            def dma_rbf(c0, c1):
                if PACK_FILTER:
                    nc.sync.dma_start(rbfT_sb[0:KF, c0:c1],
                                      rbfT_d[0:KF, c0:c1])
                    nc.sync.dma_start(rbfT_sb[32:32 + KF, c0:c1],
                                      rbfT_d[KF:2 * KF, c0:c1])
                else:
                    nc.sync.dma_start(rbfT_sb[:, c0:c1], rbfT_d[:, c0:c1])

            dma_rbf(0, head)

            xts = {}
            fpss = {}
            spsums = {}
            mlp_points = {}  # last tile index -> list of n0 chunks ready
            for n0 in range(0, A_PAD_CONST, 512):
                nsz = min(512, A_PAD_CONST - n0)
                t_req = (n0 + nsz - 1) // P
                mlp_points.setdefault(t_req, []).append(n0)

            def emit_dma(d):
                xt = xtp.tile([P, XC], BF16, name="xt", tag="xt")
                nc.sync.dma_start(xt[:], xg_d[d * P:(d + 1) * P, :])
                xts[d] = xt

            def emit_filter(g):
                if PACK_FILTER:
                    fps = fpsp.tile([P, 2, 512], F32, name="fps", tag="fps")
                else:
                    fps = fpsp.tile([P, GC], F32, name="fps", tag="fps")
                for q in range(GROUP_CHUNKS):
                    ch = g * GROUP_CHUNKS + q
                    if PACK_FILTER:
                        rg = 32 * (q % 2)
                        nc.tensor.matmul(
                            fps[:, q % 2, (q // 2) * C:(q // 2 + 1) * C],
                            lhsT=rbfT_sb[rg:rg + KF,
                                         g * 2 * P + (q // 2) * P:
                                         g * 2 * P + (q // 2 + 1) * P],
                            rhs=wrbfT_sb[rg:rg + KF, :],
                            start=True,
                            stop=True,
                            tile_position=(rg, 0),
                        )
                    else:
                        nc.tensor.matmul(
                            fps[:, q * C:(q + 1) * C],
                            lhsT=rbfT_sb[:, ch * P:(ch + 1) * P],
                            rhs=wrbfT_sb[:KF, :],
                            start=True,
                            stop=True,
                        )
                fpss[g] = fps

            def emit_consume(g):
                fps = fpss.pop(g)
                xt = xts[g // DMA_GROUPS]
                g2 = g % DMA_GROUPS
                fsb = fsbp.tile([P, GC], BF16, name="fsb", tag="fsb")
                fps_ap = fps[:] if PACK_FILTER else fps[:]
                if g % 5 == 2:
                    nc.vector.tensor_copy(fsb[:], fps_ap)
                else:
                    nc.scalar.activation(
                        fsb[:], fps_ap, mybir.ActivationFunctionType.Copy,
                    )
                msg = msgp.tile([P, GC], BF16, name="msg", tag="msg")
                nc.vector.tensor_tensor(
                    out=msg[:], in0=fsb[:],
                    in1=xt[:, g2 * GC:(g2 + 1) * GC],
                    op=mybir.AluOpType.mult,
                )
                for q in range(GROUP_CHUNKS):
                    ch = g * GROUP_CHUNKS + q
                    t, ct = divmod(ch, CPT)
                    if ct == 0:
                        spsums[t] = auxp.tile([P, 512], F32, name="spsum",
                                              tag="aux")[:, :C]
                    oh = ohp.tile([P, P], BF16, name="oh", tag="oh")
                    oh_eng = nc.vector if ch % 8 == 7 else nc.gpsimd
                    oh_eng.tensor_scalar(
                        oh[:], iota_sb[:], liT_sb[:, ch:ch + 1], None,
                        mybir.AluOpType.is_equal,
                    )
                    pq = (2 * (q % 2) + q // 2) if PACK_FILTER else q
                    nc.tensor.matmul(
                        spsums[t][:],
                        lhsT=oh[:],
                        rhs=msg[:, pq * C:(pq + 1) * C],
                        start=(ct == 0),
                        stop=(ct == CPT - 1),
                    )
                    if ct == CPT - 1:
                        if t + 1 < NT and t + 1 not in spsums:
                            spsums[t + 1] = auxp.tile(
                                [P, 512], F32, name="spsum", tag="aux")[:, :C]
                        emit_tile_end(t)
                        for n0 in mlp_points.get(t, []):
                            emit_mlp_chunk(n0)

            def emit_tile_end(t):
                nc.any.tensor_copy(h0_all[:, t * C:(t + 1) * C],
                                   spsums.pop(t)[:])
                for k in range(2):
                    tps = auxp.tile([P, P], BF16, name="tps", tag="aux")
                    nc.tensor.transpose(
                        tps[:],
                        h0_all[:, t * C + k * P: t * C + (k + 1) * P],
                        ident_sb[:],
                    )
                    nc.any.tensor_copy(hT[:, k, t * P:(t + 1) * P], tps[:])

            def emit_mlp_chunk(n0):
                nsz = min(512, A_PAD - n0)

                def layer(src_t, dst, wsb, bsb):
                    mp = auxp.tile([P, 512], F32, name="mp", tag="aux")
                    for k in range(2):
                        nc.tensor.matmul(
                            mp[:, :nsz],
                            lhsT=wsb[:, k, :] if wsb is w3T_sb
                            else wsb[:, k, 0:P],
                            rhs=src_t[:, k, n0:n0 + nsz],
                            start=(k == 0),
                            stop=(k == 1),
                        )
                    return mp

                for m in range(2):
                    mp = auxp.tile([P, 512], F32, name="mp", tag="aux")
                    for k in range(2):
                        nc.tensor.matmul(
                            mp[:, :nsz],
                            lhsT=w1T_sb[:, k, m * P:(m + 1) * P],
                            rhs=hT[:, k, n0:n0 + nsz],
                            start=(k == 0), stop=(k == 1),
                        )
                    nc.scalar.activation(
                        h1T[:, m, n0:n0 + nsz], mp[:, :nsz],
                        mybir.ActivationFunctionType.Silu,
                        bias=b1_sb[:, m:m + 1],
                    )
                for m in range(2):
                    mp = auxp.tile([P, 512], F32, name="mp", tag="aux")
                    for k in range(2):
                        nc.tensor.matmul(
                            mp[:, :nsz],
                            lhsT=w2T_sb[:, k, m * P:(m + 1) * P],
                            rhs=h1T[:, k, n0:n0 + nsz],
                            start=(k == 0), stop=(k == 1),
                        )
                    nc.scalar.activation(
                        h2T[:, m, n0:n0 + nsz], mp[:, :nsz],
                        mybir.ActivationFunctionType.Silu,
                        bias=b2_sb[:, m:m + 1],
                    )
                mp = auxp.tile([P, 512], F32, name="mp", tag="aux")
                for k in range(2):
                    nc.tensor.matmul(
                        mp[:1, :nsz],
                        lhsT=w3T_sb[:, k, :],
                        rhs=h2T[:, k, n0:n0 + nsz],
                        start=(k == 0), stop=(k == 1),
                    )
                nc.scalar.activation(
                    y_sb[:, n0:n0 + nsz], mp[:1, :nsz],
                    mybir.ActivationFunctionType.Copy, bias=b3val,
                )

            # --- pipelined emission (filter runs two groups ahead) ---
            emit_dma(0)
            emit_filter(0)
            emit_filter(1)

            # remaining constants (needed later; after the first x tile)
            nc.sync.dma_start(pf_sb[:], pf_d[:])
            if head < rbf_shape[1]:
                dma_rbf(head, rbf_shape[1])
            ident_sb = constp.tile([P, P], BF16)
            make_identity(nc, ident_sb[:])

            h0_all = pers.tile([P, NT * C], BF16)
            hT = pers.tile([P, 2, A_PAD], BF16)
            h1T = pers.tile([P, 2, A_PAD], BF16)
            h2T = pers.tile([P, 2, A_PAD], BF16)
            y_sb = pers.tile([1, A_PAD], F32)

            for g in range(G):
                if (g + 1) % DMA_GROUPS == 0 and g + 1 < G:
                    emit_dma((g + 1) // DMA_GROUPS)
                if g + 2 < G:
                    emit_filter(g + 2)
                emit_consume(g)
            nc.sync.dma_start(y_d[:], y_sb[:])

    nc.compile()
    return nc


def _prepare(x, rbf, num_atoms, edge_index_0, w_rbf, b_rbf, w1, b1, w2, b2, w3, b3):
    x = np.asarray(x, dtype=np.float32)
    rbf = np.asarray(rbf, dtype=np.float32)
    num_atoms = int(num_atoms)
    per_core, shared, dims = _host_prep(x, rbf, num_atoms, edge_index_0,
                                        np.asarray(w_rbf, np.float32),
                                        np.asarray(b_rbf, np.float32))
    wb, fb, b3val = _mlp_weights(
        np.asarray(w1, np.float32), np.asarray(b1, np.float32),
        np.asarray(w2, np.float32), np.asarray(b2, np.float32),
        np.asarray(w3, np.float32), np.asarray(b3, np.float32))
    params_bf = np.concatenate(
        [shared["params_bf"].astype(np.float32), wb], axis=1).astype(NP_BF16)
    nc = _build_bass(dims, b3val)
    in_maps = []
    for pc in per_core:
        params_f32 = np.concatenate([pc["_liT"], fb], axis=1).astype(np.float32)
        in_maps.append({"xg": pc["xg"], "rbfT": pc["rbfT"],
                        "params_bf": params_bf, "params_f32": params_f32})
    return nc, in_maps, dims


def assemble_output(res_y, dims, num_atoms):
    """res_y: list of per-core [1, A_PAD] arrays -> [num_atoms, 1]."""
    NT = dims["NT"]
    ys = np.stack([np.asarray(y)[0] for y in res_y])  # [N_CORES, A_PAD]
    b = dims["bin_of_atom"]
    out = ys[b // NT, (b % NT) * P + dims["pos_of_atom"]]
    return out.reshape(num_atoms, 1).astype(np.float32)


def kernel(**inputs) -> np.ndarray:
    num_atoms = int(inputs["num_atoms"])
    nc, in_maps, dims = _prepare(**inputs)
    res = run_bass_kernel_spmd(nc, in_maps, core_ids=list(range(N_CORES)))
    return assemble_output([r["y"] for r in res.results], dims, num_atoms)


# revision 49
# speedup vs baseline: 590.1176x; 1.0003x over previous
"""Trainium2 Bass kernel for AtomWise GNN message passing.

reference:
    rbf_filter = rbf @ w_rbf.T + b_rbf        # [E, C]
    msg = rbf_filter * x                      # [E, C]
    out = segment_sum(msg, edge_index_0, N)   # [N, C]
    out = silu(out @ w1.T + b1); out = silu(out @ w2.T + b2); out = out @ w3.T + b3

Strategy (8 NeuronCores, no collectives):
  - Host: stable-sort edges by destination atom; shard ATOMS (N/8 per core) so
    each core owns all edges of its atom range.  Within a core, atoms are
    processed in 128-atom tiles; each tile's edge list is padded to a global
    E_TILE so every core runs the identical SPMD program.
  - Device (per core, per 512-edge group):
      PE:  filter = rbf_chunk(K=17, bias row folded) @ w_rbfT -> PSUM
      ACT: evacuate filter PSUM -> SBUF bf16
      DVE: msg = filter * x  (bf16 2x mode)
      DVE: one-hot[e, a] = (iota_row == li[e])  (tensor_scalar is_equal, 4x)
      PE:  atom_psum[a, c] += one-hot.T @ msg   (scatter-add as matmul)
    Then per-atom-tile PSUM -> SBUF, PE transposes to [C, atoms] layout and a
    3-layer MLP (bf16 matmuls, f32 accumulate) runs on-chip; output [1, atoms].
"""

import os as _os

# This kernel executes on the neuron/axon PJRT devices; a JAX_PLATFORMS=cpu
# pin (meant for running jax reference oracles on CPU) would hide them.
if _os.environ.get("JAX_PLATFORMS", "") == "cpu":
    _os.environ.pop("JAX_PLATFORMS")

import numpy as np

import concourse.bacc as bacc
import concourse.mybir as mybir
import concourse.tile as tile
from concourse.bass_utils import run_bass_kernel_spmd
from concourse.masks import make_identity

N_CORES = 8
P = 128
C = 256
RBF = 16
KF = RBF + 1  # rbf channels + bias row
CHUNK = 128  # edges per scatter matmul (contraction dim)
GROUP_CHUNKS = 6
GROUP_E = CHUNK * GROUP_CHUNKS  # 768 edges per elementwise group
DMA_GROUPS = 2  # groups per x DMA (1536 edges, 0.75 MiB)
DMA_E = GROUP_E * DMA_GROUPS
BF16 = mybir.dt.bfloat16
F32 = mybir.dt.float32
NP_BF16 = mybir.dt.np(BF16)

# tile_position row-packing of the K=17 filter matmuls (2 concurrent row
# groups). Disabled: concurrent row-group matmuls draining into PSUM showed a
# rare NRT_EXEC_UNIT_UNRECOVERABLE device fault when two drains land in the
# same PSUM bank window. The serial path is deterministic and ~10% slower on
# the tensor engine only.
PACK_FILTER = False


def _host_prep(x, rbf, num_atoms, edge_index_0, w_rbf, b_rbf):
    """Sort/shard/pad on host with balanced atom binning.

    Atoms are assigned to N_CORES*NT bins (max P atoms each) by greedy LPT on
    edge count, so every bin has nearly equal edges -> minimal padding. Bin b
    maps to core b // NT, atom-tile b % NT, and an atom's one-hot column is
    its position within the bin. Returns the atom->(bin,pos) maps for output
    reassembly.
    """
    import heapq

    n_local = num_atoms // N_CORES
    assert num_atoms % N_CORES == 0
    NT = (n_local + P - 1) // P  # atom tiles per core
    NBINS = N_CORES * NT

    idx = np.asarray(edge_index_0).astype(np.int64)
    counts = np.bincount(idx, minlength=num_atoms)

    # LPT: biggest atoms first into the least-loaded non-full bin
    bin_of_atom = np.empty(num_atoms, dtype=np.int64)
    pos_of_atom = np.empty(num_atoms, dtype=np.int64)
    bin_fill = np.zeros(NBINS, dtype=np.int64)
    heap = [(0, b) for b in range(NBINS)]
    heapq.heapify(heap)
    atom_order = np.argsort(-counts, kind="stable")
    spill = []
    for a in atom_order:
        while True:
            s, b = heapq.heappop(heap)
            if bin_fill[b] < P:
                break
            spill.append((s, b))
        bin_of_atom[a] = b
        pos_of_atom[a] = bin_fill[b]
        bin_fill[b] += 1
        heapq.heappush(heap, (s + int(counts[a]), b))
        for item in spill:
            heapq.heappush(heap, item)
        spill.clear()

    bin_of_edge = bin_of_atom[idx]
    order_all = np.argsort(bin_of_edge, kind="stable")
    bin_counts = np.bincount(bin_of_edge, minlength=NBINS)
    bin_start = np.concatenate([[0], np.cumsum(bin_counts)])

    E_TILE = int(-(-bin_counts.max() // CHUNK) * CHUNK)
    while (NT * E_TILE) % GROUP_E != 0:
        E_TILE += CHUNK
    E_PAD = NT * E_TILE  # per-core consumed edge slots
    G = E_PAD // GROUP_E
    NCHUNK = E_PAD // CHUNK
    CPT = E_TILE // CHUNK  # chunks per atom tile
    D = -(-G // DMA_GROUPS)  # x DMA count (last may be partly consumed)
    E_XG = D * DMA_E

    per_core = []
    for c in range(N_CORES):
        xs = np.zeros((E_XG, C), dtype=np.float32)
        rbf_pad = np.zeros((E_PAD, KF), dtype=np.float32)
        li = np.full((E_PAD,), -1.0, dtype=np.float32)
        for t in range(NT):
            b = c * NT + t
            order = order_all[bin_start[b]:bin_start[b + 1]]
            n = len(order)
            s = t * E_TILE
            xs[s:s + n] = x[order]
            rbf_pad[s:s + n, :RBF] = rbf[order]
            rbf_pad[s:s + n, RBF] = 1.0
            li[s:s + n] = pos_of_atom[idx[order]].astype(np.float32)

        # x: [D, (4 dma-groups, 4 chunks), 128, C] -> [D*128, 16*C]
        # with PACK_FILTER, chunks within a group are stored in the psum
        # evacuation order [0, 2, 1, 3]
        xs4 = xs.reshape(D, DMA_GROUPS, GROUP_CHUNKS, P, C)
        if PACK_FILTER:
            xs4 = xs4[:, :, [0, 2, 1, 3]]
        xg = (
            xs4.reshape(D, DMA_GROUPS * GROUP_CHUNKS, P, C)
            .transpose(0, 2, 1, 3)
            .reshape(D * P, DMA_GROUPS * GROUP_CHUNKS * C)
            .astype(NP_BF16)
        )
        if PACK_FILTER:
            # rbfT packed for 2-row-group tiling: chunk (g,q) on partitions
            # [KF*(q%2), +KF) of the compact array, cols [g*256+(q//2)*128)
            arr = rbf_pad.reshape(G, GROUP_CHUNKS, P, KF)
            rbfT = np.zeros((2 * KF, G, 2, P), dtype=np.float32)
            for q in range(GROUP_CHUNKS):
                rbfT[KF * (q % 2):KF * (q % 2) + KF, :, q // 2, :] = (
                    arr[:, q].transpose(2, 0, 1)
                )
            rbfT = rbfT.reshape(2 * KF, G * 2 * P).astype(NP_BF16)
        else:
            rbfT = np.ascontiguousarray(rbf_pad.T).astype(NP_BF16)
        liT = np.ascontiguousarray(li.reshape(NCHUNK, P).T)  # f32 [P, NCHUNK]
        per_core.append({"xg": xg, "rbfT": rbfT, "_liT": liT})

    wrbfT = np.concatenate(
        [w_rbf.T.astype(np.float32), b_rbf[None].astype(np.float32)], axis=0
    )  # [KF, C]
    if PACK_FILTER:
        w4 = np.zeros((P, C), dtype=np.float32)
        for rg in (0, 32):
            w4[rg:rg + KF] = wrbfT
        wrbfT = w4
    iota = np.tile(np.arange(P, dtype=np.float32), (P, 1))
    # bf16 bundle: [iota(128) | wrbfT(C)]
    pb = np.concatenate([iota, np.zeros((P, C), np.float32)], axis=1)
    pb[:wrbfT.shape[0], P:P + C] = wrbfT
    shared = {"params_bf": pb.astype(NP_BF16)}
    dims = dict(NT=NT, A_PAD=NT * P, E_TILE=E_TILE, G=G, E_PAD=E_PAD,
                NCHUNK=NCHUNK, CPT=CPT, n_local=n_local, D=D,
                bin_of_atom=bin_of_atom, pos_of_atom=pos_of_atom)
    return per_core, shared, dims


def _mlp_weights(w1, b1, w2, b2, w3, b3):
    def wT_blocks(w):  # w [out, in] -> lhsT blocks [P, in//P, out]
        wt = w.T.astype(np.float32)  # [in, out]
        i_dim, o_dim = wt.shape
        return np.ascontiguousarray(
            wt.reshape(i_dim // P, P, o_dim).transpose(1, 0, 2)
        ).astype(NP_BF16).astype(np.float32)

    def b_blocks(b):  # [out] -> [P, out//P]
        return np.ascontiguousarray(b.astype(np.float32).reshape(-1, P).T)

    wb = np.concatenate([
        wT_blocks(w1).reshape(P, 2 * C).astype(np.float32),
        wT_blocks(w2).reshape(P, 2 * C).astype(np.float32),
        wT_blocks(w3).reshape(P, 2).astype(np.float32),
    ], axis=1)  # [P, 4C+2] -> appended to params_bf
    fb = np.concatenate([b_blocks(b1), b_blocks(b2)], axis=1)  # [P, 4]
    return wb, fb, float(np.asarray(b3).reshape(-1)[0])


def _build_bass(dims, b3val):
    NT = dims["NT"]
    A_PAD = dims["A_PAD"]
    G = dims["G"]
    E_PAD = dims["E_PAD"]
    NCHUNK = dims["NCHUNK"]
    CPT = dims["CPT"]  # chunks per atom tile
    D = dims["D"]
    A_PAD_CONST = A_PAD
    GC = GROUP_CHUNKS * C  # elementwise group width (1024)
    XC = DMA_GROUPS * GC  # x DMA tile width (4096)

    nc = bacc.Bacc("TRN2", target_bir_lowering=False, debug=False,
                   num_devices=N_CORES)
    xg_d = nc.dram_tensor("xg", [D * P, XC], BF16, kind="ExternalInput")
    rbf_shape = [2 * KF, G * 2 * P] if PACK_FILTER else [KF, E_PAD]
    rbfT_d = nc.dram_tensor("rbfT", rbf_shape, BF16, kind="ExternalInput")
    PBW = P + C + 2 * (2 * C) + 2  # iota | wrbfT | w1T | w2T | w3T
    PFW = NCHUNK + 4  # liT | b1 | b2
    pbf_d = nc.dram_tensor("params_bf", [P, PBW], BF16, kind="ExternalInput")
    pf_d = nc.dram_tensor("params_f32", [P, PFW], F32, kind="ExternalInput")
    y_d = nc.dram_tensor("y", [1, A_PAD], F32, kind="ExternalOutput")

    with tile.TileContext(nc) as tc:
        with (
            tc.tile_pool(name="const", bufs=1) as constp,
            tc.tile_pool(name="pers", bufs=1) as pers,
            tc.tile_pool(name="xt", bufs=4) as xtp,
            tc.tile_pool(name="fsb", bufs=6) as fsbp,
            tc.tile_pool(name="msg", bufs=6) as msgp,
            tc.tile_pool(name="oh", bufs=32) as ohp,
            tc.tile_pool(name="fps", bufs=2, space="PSUM") as fpsp,
            tc.tile_pool(name="aux", bufs=2, space="PSUM") as auxp,
        ):
            # --- constants: one bundled DMA each for bf16/f32 params ---
            pbf_sb = constp.tile([P, PBW], BF16)
            nc.sync.dma_start(pbf_sb[:], pbf_d[:])
            pf_sb = constp.tile([P, PFW], F32)
            iota_sb = pbf_sb[:, 0:P]
            wrbfT_sb = pbf_sb[:, P:P + C]
            w1T_sb = pbf_sb[:, P + C:P + C + 2 * C].rearrange(
                "p (k c) -> p k c", k=2)
            w2T_sb = pbf_sb[:, P + 3 * C:P + 5 * C].rearrange(
                "p (k c) -> p k c", k=2)
            w3T_sb = pbf_sb[:, P + 5 * C:P + 5 * C + 2].rearrange(
                "p (k c) -> p k c", k=2)
            liT_sb = pf_sb[:, 0:NCHUNK]
            b1_sb = pf_sb[:, NCHUNK:NCHUNK + 2]
            b2_sb = pf_sb[:, NCHUNK + 2:NCHUNK + 4]
            sb_rbf_shape = [49, rbf_shape[1]] if PACK_FILTER else rbf_shape
            rbfT_sb = constp.tile(sb_rbf_shape, BF16)
            head = min(rbf_shape[1], rbf_shape[1] // NT)

            def dma_rbf(c0, c1):
                if PACK_FILTER:
                    nc.sync.dma_start(rbfT_sb[0:KF, c0:c1],
                                      rbfT_d[0:KF, c0:c1])
                    nc.sync.dma_start(rbfT_sb[32:32 + KF, c0:c1],
                                      rbfT_d[KF:2 * KF, c0:c1])
                else:
                    nc.sync.dma_start(rbfT_sb[:, c0:c1], rbfT_d[:, c0:c1])

            dma_rbf(0, head)

            xts = {}
            fpss = {}
            spsums = {}
            mlp_points = {}  # last tile index -> list of n0 chunks ready
            for n0 in range(0, A_PAD_CONST, 512):
                nsz = min(512, A_PAD_CONST - n0)
                t_req = (n0 + nsz - 1) // P
                mlp_points.setdefault(t_req, []).append(n0)

            def emit_dma(d):
                xt = xtp.tile([P, XC], BF16, name="xt", tag="xt")
                nc.sync.dma_start(xt[:], xg_d[d * P:(d + 1) * P, :])
                xts[d] = xt

            def emit_filter(g):
                if PACK_FILTER:
                    fps = fpsp.tile([P, 2, 512], F32, name="fps", tag="fps")
                else:
                    fps = fpsp.tile([P, GC], F32, name="fps", tag="fps")
                for q in range(GROUP_CHUNKS):
                    ch = g * GROUP_CHUNKS + q
                    if PACK_FILTER:
                        rg = 32 * (q % 2)
                        nc.tensor.matmul(
                            fps[:, q % 2, (q // 2) * C:(q // 2 + 1) * C],
                            lhsT=rbfT_sb[rg:rg + KF,
                                         g * 2 * P + (q // 2) * P:
                                         g * 2 * P + (q // 2 + 1) * P],
                            rhs=wrbfT_sb[rg:rg + KF, :],
                            start=True,
                            stop=True,
                            tile_position=(rg, 0),
                        )
                    else:
                        nc.tensor.matmul(
                            fps[:, q * C:(q + 1) * C],
                            lhsT=rbfT_sb[:, ch * P:(ch + 1) * P],
                            rhs=wrbfT_sb[:KF, :],
                            start=True,
                            stop=True,
                        )
                fpss[g] = fps

            def emit_consume(g):
                fps = fpss.pop(g)
                xt = xts[g // DMA_GROUPS]
                g2 = g % DMA_GROUPS
                fsb = fsbp.tile([P, GC], BF16, name="fsb", tag="fsb")
                fps_ap = fps[:] if PACK_FILTER else fps[:]
                if g % 5 == 2:
                    nc.vector.tensor_copy(fsb[:], fps_ap)
                else:
                    nc.scalar.activation(
                        fsb[:], fps_ap, mybir.ActivationFunctionType.Copy,
                    )
                msg = msgp.tile([P, GC], BF16, name="msg", tag="msg")
                nc.vector.tensor_tensor(
                    out=msg[:], in0=fsb[:],
                    in1=xt[:, g2 * GC:(g2 + 1) * GC],
                    op=mybir.AluOpType.mult,
                )
                for q in range(GROUP_CHUNKS):
                    ch = g * GROUP_CHUNKS + q
                    t, ct = divmod(ch, CPT)
                    if ct == 0:
                        spsums[t] = auxp.tile([P, 512], F32, name="spsum",
                                              tag="aux")[:, :C]
                    oh = ohp.tile([P, P], BF16, name="oh", tag="oh")
                    oh_eng = nc.gpsimd  # all one-hots on the idle Pool engine
                    oh_eng.tensor_scalar(
                        oh[:], iota_sb[:], liT_sb[:, ch:ch + 1], None,
                        mybir.AluOpType.is_equal,
                    )
                    pq = (2 * (q % 2) + q // 2) if PACK_FILTER else q
                    nc.tensor.matmul(
                        spsums[t][:],
                        lhsT=oh[:],
                        rhs=msg[:, pq * C:(pq + 1) * C],
                        start=(ct == 0),
                        stop=(ct == CPT - 1),
                    )
                    if ct == CPT - 1:
                        if t + 1 < NT and t + 1 not in spsums:
                            spsums[t + 1] = auxp.tile(
                                [P, 512], F32, name="spsum", tag="aux")[:, :C]
                        emit_tile_end(t)
                        for n0 in mlp_points.get(t, []):
                            emit_mlp_chunk(n0)

            def emit_tile_end(t):
                nc.any.tensor_copy(h0_all[:, t * C:(t + 1) * C],
                                   spsums.pop(t)[:])
                for k in range(2):
                    tps = auxp.tile([P, P], BF16, name="tps", tag="aux")
                    nc.tensor.transpose(
                        tps[:],
                        h0_all[:, t * C + k * P: t * C + (k + 1) * P],
                        ident_sb[:],
                    )
                    nc.any.tensor_copy(hT[:, k, t * P:(t + 1) * P], tps[:])

            def emit_mlp_chunk(n0):
                nsz = min(512, A_PAD - n0)

                def layer(src_t, dst, wsb, bsb):
                    mp = auxp.tile([P, 512], F32, name="mp", tag="aux")
                    for k in range(2):
                        nc.tensor.matmul(
                            mp[:, :nsz],
                            lhsT=wsb[:, k, :] if wsb is w3T_sb
                            else wsb[:, k, 0:P],
                            rhs=src_t[:, k, n0:n0 + nsz],
                            start=(k == 0),
                            stop=(k == 1),
                        )
                    return mp

                for m in range(2):
                    mp = auxp.tile([P, 512], F32, name="mp", tag="aux")
                    for k in range(2):
                        nc.tensor.matmul(
                            mp[:, :nsz],
                            lhsT=w1T_sb[:, k, m * P:(m + 1) * P],
                            rhs=hT[:, k, n0:n0 + nsz],
                            start=(k == 0), stop=(k == 1),
                        )
                    nc.scalar.activation(
                        h1T[:, m, n0:n0 + nsz], mp[:, :nsz],
                        mybir.ActivationFunctionType.Silu,
                        bias=b1_sb[:, m:m + 1],
                    )
                for m in range(2):
                    mp = auxp.tile([P, 512], F32, name="mp", tag="aux")
                    for k in range(2):
                        nc.tensor.matmul(
                            mp[:, :nsz],
                            lhsT=w2T_sb[:, k, m * P:(m + 1) * P],
                            rhs=h1T[:, k, n0:n0 + nsz],
                            start=(k == 0), stop=(k == 1),
                        )
                    nc.scalar.activation(
                        h2T[:, m, n0:n0 + nsz], mp[:, :nsz],
                        mybir.ActivationFunctionType.Silu,
                        bias=b2_sb[:, m:m + 1],
                    )
                mp = auxp.tile([P, 512], F32, name="mp", tag="aux")
                for k in range(2):
                    nc.tensor.matmul(
                        mp[:1, :nsz],
                        lhsT=w3T_sb[:, k, :],
                        rhs=h2T[:, k, n0:n0 + nsz],
                        start=(k == 0), stop=(k == 1),
                    )
                nc.scalar.activation(
                    y_sb[:, n0:n0 + nsz], mp[:1, :nsz],
                    mybir.ActivationFunctionType.Copy, bias=b3val,
                )

            # --- pipelined emission (filter runs two groups ahead) ---
            emit_dma(0)
            emit_filter(0)
            emit_filter(1)

            # remaining constants (needed later; after the first x tile)
            nc.sync.dma_start(pf_sb[:], pf_d[:])
            if head < rbf_shape[1]:
                dma_rbf(head, rbf_shape[1])
            ident_sb = constp.tile([P, P], BF16)
            make_identity(nc, ident_sb[:])

            h0_all = pers.tile([P, NT * C], BF16)
            hT = pers.tile([P, 2, A_PAD], BF16)
            h1T = pers.tile([P, 2, A_PAD], BF16)
            h2T = pers.tile([P, 2, A_PAD], BF16)
            y_sb = pers.tile([1, A_PAD], F32)

            for g in range(G):
                if (g + 1) % DMA_GROUPS == 0 and g + 1 < G:
                    emit_dma((g + 1) // DMA_GROUPS)
                if g + 2 < G:
                    emit_filter(g + 2)
                emit_consume(g)
            nc.sync.dma_start(y_d[:], y_sb[:])

    nc.compile()
    return nc


def _prepare(x, rbf, num_atoms, edge_index_0, w_rbf, b_rbf, w1, b1, w2, b2, w3, b3):
    x = np.asarray(x, dtype=np.float32)
    rbf = np.asarray(rbf, dtype=np.float32)
    num_atoms = int(num_atoms)
    per_core, shared, dims = _host_prep(x, rbf, num_atoms, edge_index_0,
                                        np.asarray(w_rbf, np.float32),
                                        np.asarray(b_rbf, np.float32))
    wb, fb, b3val = _mlp_weights(
        np.asarray(w1, np.float32), np.asarray(b1, np.float32),
        np.asarray(w2, np.float32), np.asarray(b2, np.float32),
        np.asarray(w3, np.float32), np.asarray(b3, np.float32))
    params_bf = np.concatenate(
        [shared["params_bf"].astype(np.float32), wb], axis=1).astype(NP_BF16)
    nc = _build_bass(dims, b3val)
    in_maps = []
    for pc in per_core:
        params_f32 = np.concatenate([pc["_liT"], fb], axis=1).astype(np.float32)
        in_maps.append({"xg": pc["xg"], "rbfT": pc["rbfT"],
                        "params_bf": params_bf, "params_f32": params_f32})
    return nc, in_maps, dims


def assemble_output(res_y, dims, num_atoms):
    """res_y: list of per-core [1, A_PAD] arrays -> [num_atoms, 1]."""
    NT = dims["NT"]
    ys = np.stack([np.asarray(y)[0] for y in res_y])  # [N_CORES, A_PAD]
    b = dims["bin_of_atom"]
    out = ys[b // NT, (b % NT) * P + dims["pos_of_atom"]]
    return out.reshape(num_atoms, 1).astype(np.float32)


def kernel(**inputs) -> np.ndarray:
    num_atoms = int(inputs["num_atoms"])
    nc, in_maps, dims = _prepare(**inputs)
    res = run_bass_kernel_spmd(nc, in_maps, core_ids=list(range(N_CORES)))
    return assemble_output([r["y"] for r in res.results], dims, num_atoms)
